# revision 2
# baseline (speedup 1.0000x reference)
"""Longformer attention (B=1, S=4096, D=512, H=8, HD=64, window=512, nglobal=64)
on 8 Trainium2 NeuronCores, head-parallel (core c computes head c).

v2 design (fp16 end-to-end, fp32 PSUM accumulation):
  - Host pre-transposes inputs to xT [512, 4096] fp16.
  - qT/kT [128(d_sw|d_g), 4096] fp16 via matmul(lhsT=w[f,d], rhs=xT[f,s]);
    evacuated on ACT (q with per-partition bias; k bias is a softmax no-op
    and is skipped; v bias folds into a host-side output constant since
    softmax weights sum to 1).
  - v produced directly in natural [s, d] layout (lhsT = xT s-block,
    rhs = wv) -- no PE transposes.  Ones column appended (row-sum trick).
  - Sliding window in transposed-logits form, software-pipelined:
    logits+exp+masks of supertile t overlap AV/normalize/out-proj of t-1.
  - Softmax normalization folded into the out-projection PSUM evacuation
    as a per-partition ACT/DVE scale.
  - Global attention batched: 8 ktiles of logits per PSUM bank -> 4 exps.
  - Host sums the 8 fp16 per-head partial outputs and adds bias terms.
"""
import os
import sys
import functools

for _p in ("/opt/trn_rl_repo",):
    if os.path.isdir(_p) and _p not in sys.path:
        sys.path.insert(0, _p)

import numpy as np

import concourse.bass as bass
import concourse.tile as tile
from concourse import bacc, mybir
from concourse.bass_utils import run_bass_kernel_spmd

S = 4096
F = 512          # d_model
HD = 64          # head dim
H = 8
WIN = 512        # sliding window (left 256, right 256)
ST = 256         # query supertile
NST = S // ST    # 16
KT = 128         # key tile
NKT = S // KT    # 32
N_CORES = 8
F32 = mybir.dt.float32
F16 = mybir.dt.float16


def _build_masks(ng: int):
    """Static 0/1 masks for the transposed [k=128, q=256] logit tiles.

    For supertile t and ktile j, delta = j - 2t and d = q - k =
    qq - kk + (-delta)*128 with qq in [0,256), kk in [0,128).
    Band keeps d in [-256, 255].
    delta=-2 -> keep qq <= kk - 1;   delta=-1 -> keep qq <= kk + 127
    delta=+2 -> keep qq >= kk;       delta=+3 -> keep qq >= kk + 128
    """
    kk = np.arange(KT)[:, None]
    qq = np.arange(ST)[None, :]
    m_m2 = (qq <= kk - 1).astype(np.float16)
    m_m1 = (qq <= kk + 127).astype(np.float16)
    m_p2 = (qq >= kk).astype(np.float16)
    m_p3 = (qq >= kk + 128).astype(np.float16)
    ml = np.concatenate([m_m2, m_m1], axis=1)            # [128, 512]
    mr = np.concatenate([m_p2, m_p3], axis=1)            # [128, 512]
    m_m2g = m_m2.copy()
    if ng > 0:
        m_m2g[:ng, :] = np.float16(1.0)                  # global k rows kept
    mlg = np.concatenate([m_m2g, m_m1], axis=1)          # used at t=1
    return ml, mr, mlg


def _sw_tiles(t: int):
    """ktile range and mask placements for supertile t."""
    j0 = max(0, 2 * t - 2)
    j1 = min(NKT, 2 * t + 4)
    ml_present = 2 * t - 2 >= 0
    mr_present = 2 * t + 2 < j1
    mr_off = (2 * t + 2 - j0) * ST if mr_present else None
    return j0, j1, ml_present, mr_off


def _build_program(ng: int, use_bq: bool):
    """Build + compile the per-core bass program, specialized for ng leading
    global tokens (0 <= ng <= 128)."""
    nc = bacc.Bacc("TRN2", target_bir_lowering=False, debug=False,
                   num_devices=N_CORES)

    d = {}
    d["xqT"] = nc.dram_tensor("xqT", [F, S], F16, kind="ExternalInput").ap()
    d["xkvT"] = nc.dram_tensor("xkvT", [F, S], F16, kind="ExternalInput").ap()
    for w in ("wq", "wk", "wv"):
        d[w] = nc.dram_tensor(w, [F, 2 * HD], F16, kind="ExternalInput").ap()
    d["bq"] = nc.dram_tensor("bq", [2 * HD, 1], F32, kind="ExternalInput").ap()
    d["wo"] = nc.dram_tensor("wo", [HD, F], F16, kind="ExternalInput").ap()
    d["ml"] = nc.dram_tensor("ml", [KT, 2 * ST], F16, kind="ExternalInput").ap()
    d["mr"] = nc.dram_tensor("mr", [KT, 2 * ST], F16, kind="ExternalInput").ap()
    d["mlg"] = nc.dram_tensor("mlg", [KT, 2 * ST], F16, kind="ExternalInput").ap()
    out_ap = nc.dram_tensor("out", [S, F], F16, kind="ExternalOutput").ap()

    DC = 1024           # DMA s-chunk
    NDC = S // DC       # 4
    SC = 512            # projection s-chunk (psum bank)
    FT = F // 128       # 4 f-chunks
    Copy = mybir.ActivationFunctionType.Copy
    Exp = mybir.ActivationFunctionType.Exp

    with tile.TileContext(nc) as tc:
        with (
            tc.tile_pool(name="const", bufs=1) as constp,
            tc.tile_pool(name="big", bufs=1) as bigp,
        ):
            # ---- constants / persistent tensors ----
            wq_sb = constp.tile([128, FT, 128], F16, tag="wq")
            wk_sb = constp.tile([128, FT, 128], F16, tag="wk")
            wv_sb = constp.tile([128, FT, 128], F16, tag="wv")
            for wsb, wap in ((wq_sb, d["wq"]), (wk_sb, d["wk"]), (wv_sb, d["wv"])):
                nc.sync.dma_start(wsb[:], wap.rearrange("(c p) e -> p c e", p=128))
            bq_sb = constp.tile([128, 1], F32, tag="bq")
            if use_bq:
                nc.sync.dma_start(bq_sb[:], d["bq"][:])
            wo_sb = constp.tile([HD, F], F16, tag="wo")
            nc.sync.dma_start(wo_sb[:], d["wo"][:])
            ml_sb = constp.tile([KT, 2 * ST], F16, tag="ml")
            mr_sb = constp.tile([KT, 2 * ST], F16, tag="mr")
            mlg_sb = constp.tile([KT, 2 * ST], F16, tag="mlg")
            nc.sync.dma_start(ml_sb[:], d["ml"][:])
            nc.sync.dma_start(mr_sb[:], d["mr"][:])
            if ng > 0:
                nc.sync.dma_start(mlg_sb[:], d["mlg"][:])
            one_sb = constp.tile([128, 1], F16, tag="one")
            nc.vector.memset(one_sb[:], 1.0)

            qT = bigp.tile([128, S], F16, tag="qT")     # rows 0:64 sw, 64:128 g
            kT = bigp.tile([128, S], F16, tag="kT")
            vsw = bigp.tile([128, NKT, HD + 1], F16, tag="vsw")  # [s%128, kt, d|1]
            vg = bigp.tile([128, NKT, HD + 1], F16, tag="vg")
            nc.vector.memset(vsw[:, :, HD], 1.0)
            nc.vector.memset(vg[:, :, HD], 1.0)

            # ================= Phase A: projections =================
            with (
                tc.tile_pool(name="xin", bufs=2) as xinp,
                tc.tile_pool(name="pa", bufs=3, space="PSUM") as pap,
                tc.tile_pool(name="pv", bufs=2, space="PSUM") as pvp,
            ):
                for dc in range(NDC):
                    ds = dc * DC
                    xq_t = xinp.tile([128, FT, DC], F16, tag="xq")
                    xkv_t = xinp.tile([128, FT, DC], F16, tag="xkv")
                    nc.sync.dma_start(
                        xq_t[:], d["xqT"].rearrange("(c p) s -> p c s", p=128)[:, :, ds:ds + DC])
                    nc.sync.dma_start(
                        xkv_t[:], d["xkvT"].rearrange("(c p) s -> p c s", p=128)[:, :, ds:ds + DC])

                    for hf in range(DC // SC):
                        ss = ds + hf * SC
                        c0, c1 = hf * SC, (hf + 1) * SC
                        pq = pap.tile([128, SC], F32, tag="pa")
                        for ft in range(FT):
                            nc.tensor.matmul(pq[:], wq_sb[:, ft, :],
                                             xq_t[:, ft, c0:c1],
                                             start=(ft == 0), stop=(ft == FT - 1))
                        if use_bq:
                            nc.scalar.activation(qT[:, ss:ss + SC], pq[:], Copy,
                                                 bias=bq_sb[:, 0:1])
                        else:
                            nc.scalar.activation(qT[:, ss:ss + SC], pq[:], Copy)

                        pk = pap.tile([128, SC], F32, tag="pa")
                        for ft in range(FT):
                            nc.tensor.matmul(pk[:], wk_sb[:, ft, :],
                                             xkv_t[:, ft, c0:c1],
                                             start=(ft == 0), stop=(ft == FT - 1))
                        # k bias shifts logits by a per-query constant ->
                        # softmax-invariant; skipped exactly.
                        nc.scalar.activation(kT[:, ss:ss + SC], pk[:], Copy)

                        # v in natural [s, d] layout: lhsT = x s-block
                        pv = pvp.tile([128, 4, 128], F32, tag="pv")
                        for sb in range(4):
                            sb0 = c0 + sb * 128
                            for ft in range(FT):
                                nc.tensor.matmul(pv[:, sb, :],
                                                 xkv_t[:, ft, sb0:sb0 + 128],
                                                 wv_sb[:, ft, :],
                                                 start=(ft == 0), stop=(ft == FT - 1))
                        kt0 = ss // 128
                        nc.vector.tensor_copy(vsw[:, kt0:kt0 + 4, 0:HD],
                                              pv[:, :, 0:HD])
                        nc.vector.tensor_copy(vg[:, kt0:kt0 + 4, 0:HD],
                                              pv[:, :, HD:2 * HD])

            # ================= Phase B: global attention (rows < ng) ============
            if ng > 0:
                with (
                    tc.tile_pool(name="eg", bufs=1) as egp,
                    tc.tile_pool(name="gx", bufs=1) as gxp,
                    tc.tile_pool(name="pb", bufs=2, space="PSUM") as pbp,
                    tc.tile_pool(name="pbs", bufs=1, space="PSUM") as pbsp,
                    tc.tile_pool(name="pbx", bufs=1, space="PSUM") as pbxp,
                    tc.tile_pool(name="pbo", bufs=1, space="PSUM") as pbop,
                ):
                    GG = 512 // ng          # ktiles per psum bank
                    eg = egp.tile([128, NKT, ng], F16, tag="eg")
                    for grp in range(NKT // GG):
                        plg = pbp.tile([128, GG * ng], F32, tag="lg")
                        for j in range(GG):
                            kt = grp * GG + j
                            nc.tensor.matmul(plg[:, j * ng:(j + 1) * ng],
                                             kT[64:128, kt * KT:(kt + 1) * KT],
                                             qT[64:128, 0:ng],
                                             start=True, stop=True)
                        nc.scalar.activation(
                            eg[:, grp * GG:(grp + 1) * GG, :], plg[:], Exp,
                            scale=0.125)
                    pxg = pbxp.tile([HD + 1, ng], F32, tag="xg")
                    for kt in range(NKT):
                        nc.tensor.matmul(pxg[:], vg[:, kt, :], eg[:, kt, :],
                                         start=(kt == 0), stop=(kt == NKT - 1))
                    xgT = gxp.tile([HD + 1, ng], F16, tag="xgT")
                    nc.vector.tensor_copy(xgT[:], pxg[:])
                    psg = pbsp.tile([ng, 1], F32, tag="sg")
                    nc.tensor.matmul(psg[:], xgT[HD:HD + 1, 0:ng],
                                     one_sb[HD:HD + 1, 0:1],
                                     start=True, stop=True)
                    rg = gxp.tile([ng, 1], F32, tag="rg")
                    nc.vector.reciprocal(rg[:], psg[:])
                    pog = pbop.tile([ng, F], F32, tag="og")
                    nc.tensor.matmul(pog[:], xgT[0:HD, 0:ng], wo_sb[:],
                                     start=True, stop=True)
                    og = gxp.tile([ng, F], F16, tag="og_sb")
                    nc.scalar.activation(og[:], pog[:], Copy, scale=rg[:, 0:1])
                    nc.sync.dma_start(out_ap[0:ng, :], og[:])

            # ================= Phase C: sliding-window attention ================
            # Software-pipelined: stage1(t) = logits+exp+masks,
            # stage2(t) = AV + normalize + out-proj + store, issued one
            # supertile behind so PE never waits on ACT/DVE.
            with (
                tc.tile_pool(name="E", bufs=3) as ep,
                tc.tile_pool(name="xt", bufs=2) as xtp,
                tc.tile_pool(name="osb", bufs=2) as osbp,
                tc.tile_pool(name="rc", bufs=2) as rcp,
                tc.tile_pool(name="pL", bufs=3, space="PSUM") as pLp,
                tc.tile_pool(name="pX", bufs=2, space="PSUM") as pXp,
                tc.tile_pool(name="pS", bufs=1, space="PSUM") as pSp,
                tc.tile_pool(name="pO", bufs=1, space="PSUM") as pOp,
            ):
                Es = {}
                Egs = {}

                def stage1(t):
                    qs = t * ST
                    j0, j1, ml_present, mr_off = _sw_tiles(t)
                    nkt = j1 - j0
                    has_g = ng > 0 and j0 > 0
                    E = ep.tile([128, 6 * ST], F16, tag="E")
                    Es[t] = E
                    # 2-ktile groups: one psum bank each
                    for a in range(0, nkt, 2):
                        b = min(a + 2, nkt)
                        pl = pLp.tile([128, (b - a) * ST], F32, tag="L")
                        for s in range(b - a):
                            j = j0 + a + s
                            nc.tensor.matmul(pl[:, s * ST:(s + 1) * ST],
                                             kT[0:64, j * KT:(j + 1) * KT],
                                             qT[0:64, qs:qs + ST],
                                             start=True, stop=True)
                        nc.scalar.activation(E[:, a * ST:b * ST], pl[:], Exp,
                                             scale=0.125)
                    if has_g:
                        plg2 = pLp.tile([ng, ST], F32, tag="L")
                        nc.tensor.matmul(plg2[:], kT[0:64, 0:ng],
                                         qT[0:64, qs:qs + ST],
                                         start=True, stop=True)
                        Eg2 = ep.tile([ng, ST], F16, tag="Eg")
                        nc.scalar.activation(Eg2[:], plg2[:], Exp, scale=0.125)
                        Egs[t] = Eg2
                    # masks (ML on gpsimd, MR on vector to balance engines)
                    if ml_present:
                        msk = mlg_sb if (t == 1 and ng > 0) else ml_sb
                        nc.gpsimd.tensor_mul(E[:, 0:2 * ST], E[:, 0:2 * ST],
                                             msk[:])
                    if mr_off is not None:
                        nc.vector.tensor_mul(E[:, mr_off:mr_off + 2 * ST],
                                             E[:, mr_off:mr_off + 2 * ST],
                                             mr_sb[:])

                def stage2(t):
                    qs = t * ST
                    j0, j1, _, _ = _sw_tiles(t)
                    nkt = j1 - j0
                    has_g = ng > 0 and j0 > 0
                    E = Es.pop(t)
                    # AV: xT' = [v|1].T @ expw.T -> [65, 256], sums in row 64
                    px = pXp.tile([HD + 1, ST], F32, tag="X")
                    for s in range(nkt):
                        j = j0 + s
                        nc.tensor.matmul(px[:], vsw[:, j, :],
                                         E[:, s * ST:(s + 1) * ST],
                                         start=(s == 0),
                                         stop=(s == nkt - 1 and not has_g))
                    if has_g:
                        nc.tensor.matmul(px[:], vsw[0:ng, 0, :], Egs.pop(t),
                                         start=False, stop=True)
                    xT = xtp.tile([HD + 1, ST], F16, tag="xT")
                    nc.vector.tensor_copy(xT[:], px[:])
                    # denominators (row 64) -> per-partition columns
                    ps = pSp.tile([128, 2], F32, tag="S")
                    for hf in range(2):
                        nc.tensor.matmul(ps[:, hf:hf + 1],
                                         xT[HD:HD + 1, hf * 128:(hf + 1) * 128],
                                         one_sb[HD:HD + 1, 0:1],
                                         start=True, stop=True)
                    rc = rcp.tile([128, 2], F32, tag="rc")
                    nc.vector.reciprocal(rc[:], ps[:])
                    po = pOp.tile([128, 2, F], F32, tag="O")
                    for hf in range(2):
                        nc.tensor.matmul(po[:, hf, :],
                                         xT[0:HD, hf * 128:(hf + 1) * 128],
                                         wo_sb[:], start=True, stop=True)
                    osb = osbp.tile([128, 2, F], F16, tag="osb")
                    # normalization folded into evacuation; split DVE/ACT
                    nc.vector.tensor_scalar_mul(osb[:, 0, :], po[:, 0, :],
                                                rc[:, 0:1])
                    nc.scalar.activation(osb[:, 1, :], po[:, 1, :], Copy,
                                         scale=rc[:, 1:2])
                    if t == 0 and ng > 0:
                        nc.sync.dma_start(out_ap[ng:128, :], osb[ng:128, 0, :])
                        nc.sync.dma_start(out_ap[128:256, :], osb[:, 1, :])
                    else:
                        nc.sync.dma_start(
                            out_ap.rearrange("(a p) f -> p a f", p=128)
                            [:, 2 * t:2 * t + 2, :], osb[:])

                for step in range(NST + 1):
                    if step < NST:
                        stage1(step)
                    if step >= 1:
                        stage2(step - 1)

    nc.compile()
    return nc


@functools.lru_cache(maxsize=4)
def _get_program(ng: int, use_bq: bool):
    return _build_program(ng, use_bq)


def kernel(inputs_q, inputs_kv, global_mask,
           w_q_sw, b_q_sw, w_k_sw, b_k_sw, w_v_sw, b_v_sw,
           w_q_g, b_q_g, w_k_g, b_k_g, w_v_g, b_v_g,
           w_out, b_out,
           _trace=False, _tmpdir=None):
    gm = np.asarray(global_mask[0]).astype(bool)
    ng = int(gm.sum())
    assert gm[:ng].all() and not gm[ng:].any(), "global_mask must be a prefix mask"
    assert ng <= 128, "kernel specialized for ng <= 128"
    assert ng == 0 or 512 % ng == 0, "kernel requires ng dividing 512"

    xqT = np.ascontiguousarray(np.asarray(inputs_q[0], np.float32).T).astype(np.float16)
    xkvT = np.ascontiguousarray(np.asarray(inputs_kv[0], np.float32).T).astype(np.float16)
    ml, mr, mlg = _build_masks(ng)

    use_bq = bool(np.any(b_q_sw) or np.any(b_q_g))
    nc = _get_program(ng, use_bq)

    in_maps = []
    for h in range(N_CORES):
        wq = np.concatenate([w_q_sw[:, h, :], w_q_g[:, h, :]], axis=1).astype(np.float16)
        wk = np.concatenate([w_k_sw[:, h, :], w_k_g[:, h, :]], axis=1).astype(np.float16)
        wv = np.concatenate([w_v_sw[:, h, :], w_v_g[:, h, :]], axis=1).astype(np.float16)
        bq = np.concatenate([b_q_sw[h], b_q_g[h]]).reshape(2 * HD, 1).astype(np.float32)
        wo = np.asarray(w_out[h], np.float16)
        in_maps.append({
            "xqT": xqT, "xkvT": xkvT,
            "wq": wq, "wk": wk, "wv": wv, "bq": bq,
            "wo": wo, "ml": ml, "mr": mr, "mlg": mlg,
        })

    res = run_bass_kernel_spmd(nc, in_maps, list(range(N_CORES)),
                               trace=_trace, tmpdir=_tmpdir)
    partial = np.stack([res.results[h]["out"] for h in range(N_CORES)])
    out = partial.astype(np.float32).sum(axis=0)

    # v-bias correction: softmax weights sum to 1, so a v bias adds
    # (b_v @ w_out) summed over heads -- a constant row per branch.
    b_base = np.asarray(b_out, np.float32)
    b_sw = b_base + np.einsum("hd,hdf->f", np.asarray(b_v_sw, np.float32),
                              np.asarray(w_out, np.float32))
    b_g = b_base + np.einsum("hd,hdf->f", np.asarray(b_v_g, np.float32),
                             np.asarray(w_out, np.float32))
    out += b_sw[None, :]
    if ng > 0:
        out[:ng] += (b_g - b_sw)[None, :]
    if _trace:
        kernel._last_results = res
    return out[None].astype(np.float32)


# revision 4
# speedup vs baseline: 93.7592x; 93.7592x over previous
"""Longformer attention (B=1, S=4096, D=512, H=8, HD=64, window=512, nglobal=64)
on 8 Trainium2 NeuronCores, head-parallel (core c computes head c).

v2 design (fp16 end-to-end, fp32 PSUM accumulation):
  - Host pre-transposes inputs to xT [512, 4096] fp16.
  - qT/kT [128(d_sw|d_g), 4096] fp16 via matmul(lhsT=w[f,d], rhs=xT[f,s]);
    evacuated on ACT (q with per-partition bias; k bias is a softmax no-op
    and is skipped; v bias folds into a host-side output constant since
    softmax weights sum to 1).
  - v produced directly in natural [s, d] layout (lhsT = xT s-block,
    rhs = wv) -- no PE transposes.  Ones column appended (row-sum trick).
  - Sliding window in transposed-logits form, software-pipelined:
    logits+exp+masks of supertile t overlap AV/normalize/out-proj of t-1.
  - Softmax normalization folded into the out-projection PSUM evacuation
    as a per-partition ACT/DVE scale.
  - Global attention batched: 8 ktiles of logits per PSUM bank -> 4 exps.
  - Host sums the 8 fp16 per-head partial outputs and adds bias terms.
"""
import os
import sys
import functools

for _p in ("/opt/trn_rl_repo",):
    if os.path.isdir(_p) and _p not in sys.path:
        sys.path.insert(0, _p)

import numpy as np

import concourse.bass as bass
import concourse.tile as tile
from concourse import bacc, mybir
from concourse.bass_utils import run_bass_kernel_spmd

S = 4096
F = 512          # d_model
HD = 64          # head dim
H = 8
WIN = 512        # sliding window (left 256, right 256)
ST = 256         # query supertile
NST = S // ST    # 16
KT = 128         # key tile
NKT = S // KT    # 32
N_CORES = 8
F32 = mybir.dt.float32
F16 = mybir.dt.bfloat16  # fp16 matmul is pathologically slow on TRN2 HW
NP16 = mybir.dt.np(F16)


def _build_masks(ng: int):
    """Static 0/1 masks for the transposed [k=128, q=256] logit tiles.

    For supertile t and ktile j, delta = j - 2t and d = q - k =
    qq - kk + (-delta)*128 with qq in [0,256), kk in [0,128).
    Band keeps d in [-256, 255].
    delta=-2 -> keep qq <= kk - 1;   delta=-1 -> keep qq <= kk + 127
    delta=+2 -> keep qq >= kk;       delta=+3 -> keep qq >= kk + 128
    """
    kk = np.arange(KT)[:, None]
    qq = np.arange(ST)[None, :]
    m_m2 = (qq <= kk - 1).astype(NP16)
    m_m1 = (qq <= kk + 127).astype(NP16)
    m_p2 = (qq >= kk).astype(NP16)
    m_p3 = (qq >= kk + 128).astype(NP16)
    ml = np.concatenate([m_m2, m_m1], axis=1)            # [128, 512]
    mr = np.concatenate([m_p2, m_p3], axis=1)            # [128, 512]
    m_m2g = m_m2.copy()
    if ng > 0:
        m_m2g[:ng, :] = 1.0                        # global k rows kept
    mlg = np.concatenate([m_m2g, m_m1], axis=1)          # used at t=1
    return ml, mr, mlg


def _sw_tiles(t: int):
    """ktile range and mask placements for supertile t."""
    j0 = max(0, 2 * t - 2)
    j1 = min(NKT, 2 * t + 4)
    ml_present = 2 * t - 2 >= 0
    mr_present = 2 * t + 2 < j1
    mr_off = (2 * t + 2 - j0) * ST if mr_present else None
    return j0, j1, ml_present, mr_off


def _build_program(ng: int, use_bq: bool):
    """Build + compile the per-core bass program, specialized for ng leading
    global tokens (0 <= ng <= 128)."""
    nc = bacc.Bacc("TRN2", target_bir_lowering=False, debug=False,
                   num_devices=N_CORES)

    d = {}
    d["xqT"] = nc.dram_tensor("xqT", [F, S], F16, kind="ExternalInput").ap()
    d["xkvT"] = nc.dram_tensor("xkvT", [F, S], F16, kind="ExternalInput").ap()
    for w in ("wq", "wk", "wv"):
        d[w] = nc.dram_tensor(w, [F, 2 * HD], F16, kind="ExternalInput").ap()
    d["bq"] = nc.dram_tensor("bq", [2 * HD, 1], F32, kind="ExternalInput").ap()
    d["wo"] = nc.dram_tensor("wo", [HD, F], F16, kind="ExternalInput").ap()
    d["ml"] = nc.dram_tensor("ml", [KT, 2 * ST], F16, kind="ExternalInput").ap()
    d["mr"] = nc.dram_tensor("mr", [KT, 2 * ST], F16, kind="ExternalInput").ap()
    d["mlg"] = nc.dram_tensor("mlg", [KT, 2 * ST], F16, kind="ExternalInput").ap()
    out_ap = nc.dram_tensor("out", [S, F], F16, kind="ExternalOutput").ap()

    DC = 1024           # DMA s-chunk
    NDC = S // DC       # 4
    SC = 512            # projection s-chunk (psum bank)
    FT = F // 128       # 4 f-chunks
    Copy = mybir.ActivationFunctionType.Copy
    Exp = mybir.ActivationFunctionType.Exp

    with tile.TileContext(nc) as tc:
        with (
            tc.tile_pool(name="const", bufs=1) as constp,
            tc.tile_pool(name="big", bufs=1) as bigp,
        ):
            # ---- constants / persistent tensors ----
            wq_sb = constp.tile([128, FT, 128], F16, tag="wq")
            wk_sb = constp.tile([128, FT, 128], F16, tag="wk")
            wv_sb = constp.tile([128, FT, 128], F16, tag="wv")
            for wsb, wap in ((wq_sb, d["wq"]), (wk_sb, d["wk"]), (wv_sb, d["wv"])):
                nc.sync.dma_start(wsb[:], wap.rearrange("(c p) e -> p c e", p=128))
            bq_sb = constp.tile([128, 1], F32, tag="bq")
            if use_bq:
                nc.sync.dma_start(bq_sb[:], d["bq"][:])
            wo_sb = constp.tile([HD, F], F16, tag="wo")
            nc.sync.dma_start(wo_sb[:], d["wo"][:])
            ml_sb = constp.tile([KT, 2 * ST], F16, tag="ml")
            mr_sb = constp.tile([KT, 2 * ST], F16, tag="mr")
            mlg_sb = constp.tile([KT, 2 * ST], F16, tag="mlg")
            nc.sync.dma_start(ml_sb[:], d["ml"][:])
            nc.sync.dma_start(mr_sb[:], d["mr"][:])
            if ng > 0:
                nc.sync.dma_start(mlg_sb[:], d["mlg"][:])
            one_sb = constp.tile([128, 1], F16, tag="one")
            nc.vector.memset(one_sb[:], 1.0)

            qT = bigp.tile([128, S], F16, tag="qT")     # rows 0:64 sw, 64:128 g
            kT = bigp.tile([128, S], F16, tag="kT")
            vsw = bigp.tile([128, NKT, HD + 1], F16, tag="vsw")  # [s%128, kt, d|1]
            vg = bigp.tile([128, NKT, HD + 1], F16, tag="vg")
            nc.vector.memset(vsw[:, :, HD], 1.0)
            nc.vector.memset(vg[:, :, HD], 1.0)

            # ================= Phase A: projections =================
            with (
                tc.tile_pool(name="xin", bufs=2) as xinp,
                tc.tile_pool(name="pa", bufs=3, space="PSUM") as pap,
                tc.tile_pool(name="pv", bufs=2, space="PSUM") as pvp,
            ):
                for dc in range(NDC):
                    ds = dc * DC
                    xq_t = xinp.tile([128, FT, DC], F16, tag="xq")
                    xkv_t = xinp.tile([128, FT, DC], F16, tag="xkv")
                    nc.sync.dma_start(
                        xq_t[:], d["xqT"].rearrange("(c p) s -> p c s", p=128)[:, :, ds:ds + DC])
                    nc.sync.dma_start(
                        xkv_t[:], d["xkvT"].rearrange("(c p) s -> p c s", p=128)[:, :, ds:ds + DC])

                    for hf in range(DC // SC):
                        ss = ds + hf * SC
                        c0, c1 = hf * SC, (hf + 1) * SC
                        pq = pap.tile([128, SC], F32, tag="pa")
                        for ft in range(FT):
                            nc.tensor.matmul(pq[:], wq_sb[:, ft, :],
                                             xq_t[:, ft, c0:c1],
                                             start=(ft == 0), stop=(ft == FT - 1))
                        if use_bq:
                            nc.scalar.activation(qT[:, ss:ss + SC], pq[:], Copy,
                                                 bias=bq_sb[:, 0:1])
                        else:
                            nc.scalar.activation(qT[:, ss:ss + SC], pq[:], Copy)

                        pk = pap.tile([128, SC], F32, tag="pa")
                        for ft in range(FT):
                            nc.tensor.matmul(pk[:], wk_sb[:, ft, :],
                                             xkv_t[:, ft, c0:c1],
                                             start=(ft == 0), stop=(ft == FT - 1))
                        # k bias shifts logits by a per-query constant ->
                        # softmax-invariant; skipped exactly.
                        nc.scalar.activation(kT[:, ss:ss + SC], pk[:], Copy)

                        # v in natural [s, d] layout: lhsT = x s-block
                        pv = pvp.tile([128, 4, 128], F32, tag="pv")
                        for sb in range(4):
                            sb0 = c0 + sb * 128
                            for ft in range(FT):
                                nc.tensor.matmul(pv[:, sb, :],
                                                 xkv_t[:, ft, sb0:sb0 + 128],
                                                 wv_sb[:, ft, :],
                                                 start=(ft == 0), stop=(ft == FT - 1))
                        kt0 = ss // 128
                        nc.vector.tensor_copy(vsw[:, kt0:kt0 + 4, 0:HD],
                                              pv[:, :, 0:HD])
                        nc.vector.tensor_copy(vg[:, kt0:kt0 + 4, 0:HD],
                                              pv[:, :, HD:2 * HD])

            # ================= Phase B: global attention (rows < ng) ============
            if ng > 0:
                with (
                    tc.tile_pool(name="eg", bufs=1) as egp,
                    tc.tile_pool(name="gx", bufs=1) as gxp,
                    tc.tile_pool(name="pb", bufs=2, space="PSUM") as pbp,
                    tc.tile_pool(name="pbs", bufs=1, space="PSUM") as pbsp,
                    tc.tile_pool(name="pbx", bufs=1, space="PSUM") as pbxp,
                    tc.tile_pool(name="pbo", bufs=1, space="PSUM") as pbop,
                ):
                    GG = 512 // ng          # ktiles per psum bank
                    eg = egp.tile([128, NKT, ng], F16, tag="eg")
                    for grp in range(NKT // GG):
                        plg = pbp.tile([128, GG * ng], F32, tag="lg")
                        for j in range(GG):
                            kt = grp * GG + j
                            nc.tensor.matmul(plg[:, j * ng:(j + 1) * ng],
                                             kT[64:128, kt * KT:(kt + 1) * KT],
                                             qT[64:128, 0:ng],
                                             start=True, stop=True)
                        nc.scalar.activation(
                            eg[:, grp * GG:(grp + 1) * GG, :], plg[:], Exp,
                            scale=0.125)
                    pxg = pbxp.tile([HD + 1, ng], F32, tag="xg")
                    for kt in range(NKT):
                        nc.tensor.matmul(pxg[:], vg[:, kt, :], eg[:, kt, :],
                                         start=(kt == 0), stop=(kt == NKT - 1))
                    xgT = gxp.tile([HD + 1, ng], F16, tag="xgT")
                    nc.vector.tensor_copy(xgT[:], pxg[:])
                    psg = pbsp.tile([ng, 1], F32, tag="sg")
                    nc.tensor.matmul(psg[:], xgT[HD:HD + 1, 0:ng],
                                     one_sb[HD:HD + 1, 0:1],
                                     start=True, stop=True)
                    rg = gxp.tile([ng, 1], F32, tag="rg")
                    nc.vector.reciprocal(rg[:], psg[:])
                    pog = pbop.tile([ng, F], F32, tag="og")
                    nc.tensor.matmul(pog[:], xgT[0:HD, 0:ng], wo_sb[:],
                                     start=True, stop=True)
                    og = gxp.tile([ng, F], F16, tag="og_sb")
                    nc.scalar.activation(og[:], pog[:], Copy, scale=rg[:, 0:1])
                    nc.sync.dma_start(out_ap[0:ng, :], og[:])

            # ================= Phase C: sliding-window attention ================
            # Software-pipelined: stage1(t) = logits+exp+masks,
            # stage2(t) = AV + normalize + out-proj + store, issued one
            # supertile behind so PE never waits on ACT/DVE.
            with (
                tc.tile_pool(name="E", bufs=3) as ep,
                tc.tile_pool(name="xt", bufs=2) as xtp,
                tc.tile_pool(name="osb", bufs=2) as osbp,
                tc.tile_pool(name="rc", bufs=2) as rcp,
                tc.tile_pool(name="pL", bufs=3, space="PSUM") as pLp,
                tc.tile_pool(name="pX", bufs=2, space="PSUM") as pXp,
                tc.tile_pool(name="pS", bufs=1, space="PSUM") as pSp,
                tc.tile_pool(name="pO", bufs=1, space="PSUM") as pOp,
            ):
                Es = {}
                Egs = {}

                def stage1(t):
                    qs = t * ST
                    j0, j1, ml_present, mr_off = _sw_tiles(t)
                    nkt = j1 - j0
                    has_g = ng > 0 and j0 > 0
                    E = ep.tile([128, 6 * ST], F16, tag="E")
                    Es[t] = E
                    # 2-ktile groups: one psum bank each
                    for a in range(0, nkt, 2):
                        b = min(a + 2, nkt)
                        pl = pLp.tile([128, (b - a) * ST], F32, tag="L")
                        for s in range(b - a):
                            j = j0 + a + s
                            nc.tensor.matmul(pl[:, s * ST:(s + 1) * ST],
                                             kT[0:64, j * KT:(j + 1) * KT],
                                             qT[0:64, qs:qs + ST],
                                             start=True, stop=True)
                        nc.scalar.activation(E[:, a * ST:b * ST], pl[:], Exp,
                                             scale=0.125)
                    if has_g:
                        plg2 = pLp.tile([ng, ST], F32, tag="L")
                        nc.tensor.matmul(plg2[:], kT[0:64, 0:ng],
                                         qT[0:64, qs:qs + ST],
                                         start=True, stop=True)
                        Eg2 = ep.tile([ng, ST], F16, tag="Eg")
                        nc.scalar.activation(Eg2[:], plg2[:], Exp, scale=0.125)
                        Egs[t] = Eg2
                    # masks (ML on gpsimd, MR on vector to balance engines)
                    if ml_present:
                        msk = mlg_sb if (t == 1 and ng > 0) else ml_sb
                        nc.gpsimd.tensor_mul(E[:, 0:2 * ST], E[:, 0:2 * ST],
                                             msk[:])
                    if mr_off is not None:
                        nc.vector.tensor_mul(E[:, mr_off:mr_off + 2 * ST],
                                             E[:, mr_off:mr_off + 2 * ST],
                                             mr_sb[:])

                def stage2(t):
                    qs = t * ST
                    j0, j1, _, _ = _sw_tiles(t)
                    nkt = j1 - j0
                    has_g = ng > 0 and j0 > 0
                    E = Es.pop(t)
                    # AV: xT' = [v|1].T @ expw.T -> [65, 256], sums in row 64
                    px = pXp.tile([HD + 1, ST], F32, tag="X")
                    for s in range(nkt):
                        j = j0 + s
                        nc.tensor.matmul(px[:], vsw[:, j, :],
                                         E[:, s * ST:(s + 1) * ST],
                                         start=(s == 0),
                                         stop=(s == nkt - 1 and not has_g))
                    if has_g:
                        nc.tensor.matmul(px[:], vsw[0:ng, 0, :], Egs.pop(t),
                                         start=False, stop=True)
                    xT = xtp.tile([HD + 1, ST], F16, tag="xT")
                    nc.vector.tensor_copy(xT[:], px[:])
                    # denominators (row 64) -> per-partition columns
                    ps = pSp.tile([128, 2], F32, tag="S")
                    for hf in range(2):
                        nc.tensor.matmul(ps[:, hf:hf + 1],
                                         xT[HD:HD + 1, hf * 128:(hf + 1) * 128],
                                         one_sb[HD:HD + 1, 0:1],
                                         start=True, stop=True)
                    rc = rcp.tile([128, 2], F32, tag="rc")
                    nc.vector.reciprocal(rc[:], ps[:])
                    po = pOp.tile([128, 2, F], F32, tag="O")
                    for hf in range(2):
                        nc.tensor.matmul(po[:, hf, :],
                                         xT[0:HD, hf * 128:(hf + 1) * 128],
                                         wo_sb[:], start=True, stop=True)
                    osb = osbp.tile([128, 2, F], F16, tag="osb")
                    # normalization folded into evacuation; split DVE/ACT
                    nc.vector.tensor_scalar_mul(osb[:, 0, :], po[:, 0, :],
                                                rc[:, 0:1])
                    nc.scalar.activation(osb[:, 1, :], po[:, 1, :], Copy,
                                         scale=rc[:, 1:2])
                    if t == 0 and ng > 0:
                        nc.sync.dma_start(out_ap[ng:128, :], osb[ng:128, 0, :])
                        nc.sync.dma_start(out_ap[128:256, :], osb[:, 1, :])
                    else:
                        nc.sync.dma_start(
                            out_ap.rearrange("(a p) f -> p a f", p=128)
                            [:, 2 * t:2 * t + 2, :], osb[:])

                for step in range(NST + 1):
                    if step < NST:
                        stage1(step)
                    if step >= 1:
                        stage2(step - 1)

    nc.compile()
    return nc


@functools.lru_cache(maxsize=4)
def _get_program(ng: int, use_bq: bool):
    return _build_program(ng, use_bq)


def kernel(inputs_q, inputs_kv, global_mask,
           w_q_sw, b_q_sw, w_k_sw, b_k_sw, w_v_sw, b_v_sw,
           w_q_g, b_q_g, w_k_g, b_k_g, w_v_g, b_v_g,
           w_out, b_out,
           _trace=False, _tmpdir=None):
    gm = np.asarray(global_mask[0]).astype(bool)
    ng = int(gm.sum())
    assert gm[:ng].all() and not gm[ng:].any(), "global_mask must be a prefix mask"
    assert ng <= 128, "kernel specialized for ng <= 128"
    assert ng == 0 or 512 % ng == 0, "kernel requires ng dividing 512"

    xqT = np.ascontiguousarray(np.asarray(inputs_q[0], np.float32).T).astype(NP16)
    xkvT = np.ascontiguousarray(np.asarray(inputs_kv[0], np.float32).T).astype(NP16)
    ml, mr, mlg = _build_masks(ng)

    use_bq = bool(np.any(b_q_sw) or np.any(b_q_g))
    nc = _get_program(ng, use_bq)

    in_maps = []
    for h in range(N_CORES):
        wq = np.concatenate([w_q_sw[:, h, :], w_q_g[:, h, :]], axis=1).astype(NP16)
        wk = np.concatenate([w_k_sw[:, h, :], w_k_g[:, h, :]], axis=1).astype(NP16)
        wv = np.concatenate([w_v_sw[:, h, :], w_v_g[:, h, :]], axis=1).astype(NP16)
        bq = np.concatenate([b_q_sw[h], b_q_g[h]]).reshape(2 * HD, 1).astype(np.float32)
        wo = np.asarray(w_out[h], NP16)
        in_maps.append({
            "xqT": xqT, "xkvT": xkvT,
            "wq": wq, "wk": wk, "wv": wv, "bq": bq,
            "wo": wo, "ml": ml, "mr": mr, "mlg": mlg,
        })

    res = run_bass_kernel_spmd(nc, in_maps, list(range(N_CORES)),
                               trace=_trace, tmpdir=_tmpdir)
    partial = np.stack([res.results[h]["out"] for h in range(N_CORES)])
    out = partial.astype(np.float32).sum(axis=0)

    # v-bias correction: softmax weights sum to 1, so a v bias adds
    # (b_v @ w_out) summed over heads -- a constant row per branch.
    b_base = np.asarray(b_out, np.float32)
    b_sw = b_base + np.einsum("hd,hdf->f", np.asarray(b_v_sw, np.float32),
                              np.asarray(w_out, np.float32))
    b_g = b_base + np.einsum("hd,hdf->f", np.asarray(b_v_g, np.float32),
                             np.asarray(w_out, np.float32))
    out += b_sw[None, :]
    if ng > 0:
        out[:ng] += (b_g - b_sw)[None, :]
    if _trace:
        kernel._last_results = res
    return out[None].astype(np.float32)


# revision 13
# speedup vs baseline: 98.9769x; 1.0557x over previous
"""Longformer attention (B=1, S=4096, D=512, H=8, HD=64, window=512, nglobal=64)
on 8 Trainium2 NeuronCores, head-parallel (core c computes head c).

v2 design (fp16 end-to-end, fp32 PSUM accumulation):
  - Host pre-transposes inputs to xT [512, 4096] fp16.
  - qT/kT [128(d_sw|d_g), 4096] fp16 via matmul(lhsT=w[f,d], rhs=xT[f,s]);
    evacuated on ACT (q with per-partition bias; k bias is a softmax no-op
    and is skipped; v bias folds into a host-side output constant since
    softmax weights sum to 1).
  - v produced directly in natural [s, d] layout (lhsT = xT s-block,
    rhs = wv) -- no PE transposes.  Ones column appended (row-sum trick).
  - Sliding window in transposed-logits form, software-pipelined:
    logits+exp+masks of supertile t overlap AV/normalize/out-proj of t-1.
  - Softmax normalization folded into the out-projection PSUM evacuation
    as a per-partition ACT/DVE scale.
  - Global attention batched: 8 ktiles of logits per PSUM bank -> 4 exps.
  - Host sums the 8 fp16 per-head partial outputs and adds bias terms.
"""
import os
import sys
import functools

for _p in ("/opt/trn_rl_repo",):
    if os.path.isdir(_p) and _p not in sys.path:
        sys.path.insert(0, _p)

import numpy as np

import concourse.bass as bass
import concourse.tile as tile
from concourse import bacc, mybir
from concourse.bass_utils import run_bass_kernel_spmd

S = 4096
F = 512          # d_model
HD = 64          # head dim
H = 8
WIN = 512        # sliding window (left 256, right 256)
ST = 256         # query supertile
NST = S // ST    # 16
KT = 128         # key tile
NKT = S // KT    # 32
N_CORES = 8
F32 = mybir.dt.float32
F16 = mybir.dt.bfloat16  # fp16 matmul is pathologically slow on TRN2 HW
NP16 = mybir.dt.np(F16)


def _build_masks(ng: int):
    """Static 0/1 masks for the transposed [k=128, q=256] logit tiles.

    For supertile t and ktile j, delta = j - 2t and d = q - k =
    qq - kk + (-delta)*128 with qq in [0,256), kk in [0,128).
    Band keeps d in [-256, 255].
    delta=-2 -> keep qq <= kk - 1;   delta=-1 -> keep qq <= kk + 127
    delta=+2 -> keep qq >= kk;       delta=+3 -> keep qq >= kk + 128
    """
    kk = np.arange(KT)[:, None]
    qq = np.arange(ST)[None, :]
    m_m2 = (qq <= kk - 1).astype(NP16)
    m_m1 = (qq <= kk + 127).astype(NP16)
    m_p2 = (qq >= kk).astype(NP16)
    m_p3 = (qq >= kk + 128).astype(NP16)
    ml = np.concatenate([m_m2, m_m1], axis=1)            # [128, 512]
    mr = np.concatenate([m_p2, m_p3], axis=1)            # [128, 512]
    m_m2g = m_m2.copy()
    if ng > 0:
        m_m2g[:ng, :] = 1.0                        # global k rows kept
    mlg = np.concatenate([m_m2g, m_m1], axis=1)          # used at t=1
    return ml, mr, mlg


def _sw_tiles(t: int):
    """ktile range and mask placements for supertile t."""
    j0 = max(0, 2 * t - 2)
    j1 = min(NKT, 2 * t + 4)
    ml_present = 2 * t - 2 >= 0
    mr_present = 2 * t + 2 < j1
    mr_off = (2 * t + 2 - j0) * ST if mr_present else None
    return j0, j1, ml_present, mr_off


def _build_program(ng: int, use_bq: bool):
    """Build + compile the per-core bass program, specialized for ng leading
    global tokens (0 <= ng <= 128)."""
    nc = bacc.Bacc("TRN2", target_bir_lowering=False, debug=False,
                   num_devices=N_CORES)

    d = {}
    d["xqT"] = nc.dram_tensor("xqT", [F, S], F16, kind="ExternalInput").ap()
    d["xkvT"] = nc.dram_tensor("xkvT", [F, S], F16, kind="ExternalInput").ap()
    for w in ("wq", "wk", "wv"):
        d[w] = nc.dram_tensor(w, [F, 2 * HD], F16, kind="ExternalInput").ap()
    d["bq"] = nc.dram_tensor("bq", [2 * HD, 1], F32, kind="ExternalInput").ap()
    d["wo"] = nc.dram_tensor("wo", [HD, F], F16, kind="ExternalInput").ap()
    d["ml"] = nc.dram_tensor("ml", [KT, 2 * ST], F16, kind="ExternalInput").ap()
    d["mr"] = nc.dram_tensor("mr", [KT, 2 * ST], F16, kind="ExternalInput").ap()
    d["mlg"] = nc.dram_tensor("mlg", [KT, 2 * ST], F16, kind="ExternalInput").ap()
    out_ap = nc.dram_tensor("out", [S, F], F16, kind="ExternalOutput").ap()

    DC = 1024           # DMA s-chunk
    NDC = S // DC       # 4
    SC = 512            # projection s-chunk (psum bank)
    FT = F // 128       # 4 f-chunks
    Copy = mybir.ActivationFunctionType.Copy
    Exp = mybir.ActivationFunctionType.Exp

    with tile.TileContext(nc) as tc:
        with (
            tc.tile_pool(name="const", bufs=1) as constp,
            tc.tile_pool(name="big", bufs=1) as bigp,
        ):
            # ---- constants / persistent tensors ----
            wq_sb = constp.tile([128, FT, 128], F16, tag="wq")
            wk_sb = constp.tile([128, FT, 128], F16, tag="wk")
            wv_sb = constp.tile([128, FT, 128], F16, tag="wv")
            for wsb, wap in ((wq_sb, d["wq"]), (wk_sb, d["wk"]), (wv_sb, d["wv"])):
                nc.sync.dma_start(wsb[:], wap.rearrange("(c p) e -> p c e", p=128))
            bq_sb = constp.tile([128, 1], F32, tag="bq")
            if use_bq:
                nc.sync.dma_start(bq_sb[:], d["bq"][:])
            wo_sb = constp.tile([HD, F], F16, tag="wo")
            ml_sb = constp.tile([KT, 2 * ST], F16, tag="ml")
            mr_sb = constp.tile([KT, 2 * ST], F16, tag="mr")
            mlg_sb = constp.tile([KT, 2 * ST], F16, tag="mlg")
            one_sb = constp.tile([128, 1], F16, tag="one")
            nc.vector.memset(one_sb[:], 1.0)

            qT = bigp.tile([128, S], F16, tag="qT")     # rows 0:64 sw, 64:128 g
            kT = bigp.tile([128, S], F16, tag="kT")
            vsw = bigp.tile([128, NKT, HD + 1], F16, tag="vsw")  # [s%128, kt, d|1]
            vg = bigp.tile([128, NKT, HD + 1], F16, tag="vg")
            nc.vector.memset(vsw[:, :, HD], 1.0)
            nc.vector.memset(vg[:, :, HD], 1.0)

            # ================= Phase A: projections =================
            with (
                tc.tile_pool(name="xin", bufs=2) as xinp,
                tc.tile_pool(name="pa", bufs=3, space="PSUM") as pap,
                tc.tile_pool(name="pv", bufs=2, space="PSUM") as pvp,
            ):
                for dc in range(NDC):
                    ds = dc * DC
                    xq_t = xinp.tile([128, FT, DC], F16, tag="xq")
                    xkv_t = xinp.tile([128, FT, DC], F16, tag="xkv")
                    nc.sync.dma_start(
                        xq_t[:], d["xqT"].rearrange("(c p) s -> p c s", p=128)[:, :, ds:ds + DC])
                    nc.sync.dma_start(
                        xkv_t[:], d["xkvT"].rearrange("(c p) s -> p c s", p=128)[:, :, ds:ds + DC])
                    if dc == 0:
                        # masks/wo are not needed until phase C; issue their
                        # loads after the first input chunk so they don't
                        # delay the first matmuls.
                        nc.sync.dma_start(ml_sb[:], d["ml"][:])
                        nc.sync.dma_start(mr_sb[:], d["mr"][:])
                        if ng > 0:
                            nc.sync.dma_start(mlg_sb[:], d["mlg"][:])
                        nc.sync.dma_start(wo_sb[:], d["wo"][:])

                    for hf in range(DC // SC):
                        ss = ds + hf * SC
                        c0, c1 = hf * SC, (hf + 1) * SC
                        pq = pap.tile([128, SC], F32, tag="pa")
                        for ft in range(FT):
                            nc.tensor.matmul(pq[:], wq_sb[:, ft, :],
                                             xq_t[:, ft, c0:c1],
                                             start=(ft == 0), stop=(ft == FT - 1))
                        if use_bq:
                            nc.scalar.activation(qT[:, ss:ss + SC], pq[:], Copy,
                                                 bias=bq_sb[:, 0:1])
                        else:
                            nc.scalar.activation(qT[:, ss:ss + SC], pq[:], Copy)

                        pk = pap.tile([128, SC], F32, tag="pa")
                        for ft in range(FT):
                            nc.tensor.matmul(pk[:], wk_sb[:, ft, :],
                                             xkv_t[:, ft, c0:c1],
                                             start=(ft == 0), stop=(ft == FT - 1))
                        # k bias shifts logits by a per-query constant ->
                        # softmax-invariant; skipped exactly.
                        nc.scalar.activation(kT[:, ss:ss + SC], pk[:], Copy)

                        # v in natural [s, d] layout: lhsT = x s-block
                        pv = pvp.tile([128, 4, 128], F32, tag="pv")
                        for sb in range(4):
                            sb0 = c0 + sb * 128
                            for ft in range(FT):
                                nc.tensor.matmul(pv[:, sb, :],
                                                 xkv_t[:, ft, sb0:sb0 + 128],
                                                 wv_sb[:, ft, :],
                                                 start=(ft == 0), stop=(ft == FT - 1))
                        kt0 = ss // 128
                        nc.vector.tensor_copy(vsw[:, kt0:kt0 + 4, 0:HD],
                                              pv[:, :, 0:HD])
                        nc.vector.tensor_copy(vg[:, kt0:kt0 + 4, 0:HD],
                                              pv[:, :, HD:2 * HD])

            # ================= Phase B: global attention (rows < ng) ============
            if ng > 0:
                with (
                    tc.tile_pool(name="eg", bufs=1) as egp,
                    tc.tile_pool(name="gx", bufs=1) as gxp,
                    tc.tile_pool(name="pb", bufs=2, space="PSUM") as pbp,
                    tc.tile_pool(name="pbs", bufs=1, space="PSUM") as pbsp,
                    tc.tile_pool(name="pbx", bufs=1, space="PSUM") as pbxp,
                    tc.tile_pool(name="pbo", bufs=1, space="PSUM") as pbop,
                ):
                    GG = 512 // ng          # ktiles per psum bank
                    NGRP = NKT // GG
                    eg = egp.tile([128, NKT, ng], F16, tag="eg")
                    pxg = pbxp.tile([HD + 1, ng], F32, tag="xg")
                    # pipeline: logits+exp of group g overlap AV of group g-1
                    for grp in range(NGRP + 1):
                        if grp < NGRP:
                            plg = pbp.tile([128, GG * ng], F32, tag="lg")
                            for j in range(GG):
                                kt = grp * GG + j
                                nc.tensor.matmul(plg[:, j * ng:(j + 1) * ng],
                                                 kT[64:128, kt * KT:(kt + 1) * KT],
                                                 qT[64:128, 0:ng],
                                                 start=True, stop=True)
                            nc.scalar.activation(
                                eg[:, grp * GG:(grp + 1) * GG, :], plg[:], Exp,
                                scale=0.125)
                        if grp >= 1:
                            for j in range(GG):
                                kt = (grp - 1) * GG + j
                                nc.tensor.matmul(pxg[:], vg[:, kt, :],
                                                 eg[:, kt, :],
                                                 start=(kt == 0),
                                                 stop=(kt == NKT - 1))
                    xgT = gxp.tile([HD + 1, ng], F16, tag="xgT")
                    nc.vector.tensor_copy(xgT[:], pxg[:])
                    psg = pbsp.tile([ng, 1], F32, tag="sg")
                    nc.tensor.matmul(psg[:], xgT[HD:HD + 1, 0:ng],
                                     one_sb[HD:HD + 1, 0:1],
                                     start=True, stop=True)
                    rg = gxp.tile([ng, 1], F32, tag="rg")
                    nc.vector.reciprocal(rg[:], psg[:])
                    pog = pbop.tile([ng, F], F32, tag="og")
                    nc.tensor.matmul(pog[:], xgT[0:HD, 0:ng], wo_sb[:],
                                     start=True, stop=True)
                    og = gxp.tile([ng, F], F16, tag="og_sb")
                    nc.scalar.activation(og[:], pog[:], Copy, scale=rg[:, 0:1])
                    nc.sync.dma_start(out_ap[0:ng, :], og[:])

            # ================= Phase C: sliding-window attention ================
            # 3-stage software pipeline: stage1(t) = logits+exp+masks,
            # stage2(t) = AV + psum->sbuf cast, stage3(t) = denominators +
            # reciprocal + out-proj + normalize-evacuate + store.  Issued with
            # skew so PE never waits on ACT/DVE results of the same supertile.
            with (
                tc.tile_pool(name="E", bufs=4) as ep,
                tc.tile_pool(name="xt", bufs=3) as xtp,
                tc.tile_pool(name="osb", bufs=2) as osbp,
                tc.tile_pool(name="rc", bufs=2) as rcp,
                tc.tile_pool(name="pL", bufs=3, space="PSUM") as pLp,
                tc.tile_pool(name="pX", bufs=2, space="PSUM") as pXp,
                tc.tile_pool(name="pS", bufs=1, space="PSUM") as pSp,
                tc.tile_pool(name="pO", bufs=1, space="PSUM") as pOp,
            ):
                Es = {}
                Egs = {}
                xts = {}

                def stage1(t):
                    qs = t * ST
                    j0, j1, ml_present, mr_off = _sw_tiles(t)
                    nkt = j1 - j0
                    has_g = ng > 0 and j0 > 0
                    E = ep.tile([128, 6 * ST], F16, tag="E")
                    Es[t] = E
                    # global-key prepend first: its small exp frees the pL
                    # slot early, keeping the pool at bufs=3 stall-free
                    if has_g:
                        plg2 = pLp.tile([ng, ST], F32, tag="L")
                        nc.tensor.matmul(plg2[:], kT[0:64, 0:ng],
                                         qT[0:64, qs:qs + ST],
                                         start=True, stop=True)
                        Eg2 = ep.tile([ng, ST], F16, tag="Eg")
                        nc.scalar.activation(Eg2[:], plg2[:], Exp, scale=0.125)
                        Egs[t] = Eg2
                    # 2-ktile groups: one psum bank each
                    for a in range(0, nkt, 2):
                        b = min(a + 2, nkt)
                        pl = pLp.tile([128, (b - a) * ST], F32, tag="L")
                        for s in range(b - a):
                            j = j0 + a + s
                            nc.tensor.matmul(pl[:, s * ST:(s + 1) * ST],
                                             kT[0:64, j * KT:(j + 1) * KT],
                                             qT[0:64, qs:qs + ST],
                                             start=True, stop=True)
                        nc.scalar.activation(E[:, a * ST:b * ST], pl[:], Exp,
                                             scale=0.125)
                    # masks (ML on gpsimd, MR on vector to balance engines)
                    if ml_present:
                        msk = mlg_sb if (t == 1 and ng > 0) else ml_sb
                        nc.gpsimd.tensor_mul(E[:, 0:2 * ST], E[:, 0:2 * ST],
                                             msk[:])
                    if mr_off is not None:
                        nc.vector.tensor_mul(E[:, mr_off:mr_off + 2 * ST],
                                             E[:, mr_off:mr_off + 2 * ST],
                                             mr_sb[:])

                def stage2(t):
                    j0, j1, _, _ = _sw_tiles(t)
                    nkt = j1 - j0
                    has_g = ng > 0 and j0 > 0
                    E = Es.pop(t)
                    # AV: xT' = [v|1].T @ expw.T -> [65, 256], sums in row 64
                    px = pXp.tile([HD + 1, ST], F32, tag="X")
                    for s in range(nkt):
                        j = j0 + s
                        nc.tensor.matmul(px[:], vsw[:, j, :],
                                         E[:, s * ST:(s + 1) * ST],
                                         start=(s == 0),
                                         stop=(s == nkt - 1 and not has_g))
                    if has_g:
                        nc.tensor.matmul(px[:], vsw[0:ng, 0, :], Egs.pop(t),
                                         start=False, stop=True)
                    xT = xtp.tile([HD + 1, ST], F16, tag="xT")
                    nc.vector.tensor_copy(xT[:], px[:])
                    xts[t] = xT

                def stage3(t):
                    xT = xts.pop(t)
                    # denominators (row 64) -> per-partition columns
                    ps = pSp.tile([128, 2], F32, tag="S")
                    for hf in range(2):
                        nc.tensor.matmul(ps[:, hf:hf + 1],
                                         xT[HD:HD + 1, hf * 128:(hf + 1) * 128],
                                         one_sb[HD:HD + 1, 0:1],
                                         start=True, stop=True)
                    rc = rcp.tile([128, 2], F32, tag="rc")
                    nc.vector.reciprocal(rc[:], ps[:])
                    po = pOp.tile([128, 2, F], F32, tag="O")
                    for hf in range(2):
                        nc.tensor.matmul(po[:, hf, :],
                                         xT[0:HD, hf * 128:(hf + 1) * 128],
                                         wo_sb[:], start=True, stop=True)
                    osb = osbp.tile([128, 2, F], F16, tag="osb")
                    # normalization folded into the evacuation copies
                    nc.vector.tensor_scalar_mul(osb[:, 0, :], po[:, 0, :],
                                                rc[:, 0:1])
                    nc.vector.tensor_scalar_mul(osb[:, 1, :], po[:, 1, :],
                                                rc[:, 1:2])
                    if t == 0 and ng > 0:
                        nc.sync.dma_start(out_ap[ng:128, :], osb[ng:128, 0, :])
                        nc.sync.dma_start(out_ap[128:256, :], osb[:, 1, :])
                    else:
                        nc.sync.dma_start(
                            out_ap.rearrange("(a p) f -> p a f", p=128)
                            [:, 2 * t:2 * t + 2, :], osb[:])

                for step in range(NST + 2):
                    if step < NST:
                        stage1(step)
                    if 1 <= step <= NST:
                        stage2(step - 1)
                    if step >= 2:
                        stage3(step - 2)

    nc.compile()
    return nc


@functools.lru_cache(maxsize=4)
def _get_program(ng: int, use_bq: bool):
    return _build_program(ng, use_bq)


def kernel(inputs_q, inputs_kv, global_mask,
           w_q_sw, b_q_sw, w_k_sw, b_k_sw, w_v_sw, b_v_sw,
           w_q_g, b_q_g, w_k_g, b_k_g, w_v_g, b_v_g,
           w_out, b_out,
           _trace=False, _tmpdir=None):
    gm = np.asarray(global_mask[0]).astype(bool)
    ng = int(gm.sum())
    assert gm[:ng].all() and not gm[ng:].any(), "global_mask must be a prefix mask"
    assert ng <= 128, "kernel specialized for ng <= 128"
    assert ng == 0 or 512 % ng == 0, "kernel requires ng dividing 512"

    xqT = np.ascontiguousarray(np.asarray(inputs_q[0], np.float32).T).astype(NP16)
    xkvT = np.ascontiguousarray(np.asarray(inputs_kv[0], np.float32).T).astype(NP16)
    ml, mr, mlg = _build_masks(ng)

    use_bq = bool(np.any(b_q_sw) or np.any(b_q_g))
    nc = _get_program(ng, use_bq)

    in_maps = []
    for h in range(N_CORES):
        wq = np.concatenate([w_q_sw[:, h, :], w_q_g[:, h, :]], axis=1).astype(NP16)
        wk = np.concatenate([w_k_sw[:, h, :], w_k_g[:, h, :]], axis=1).astype(NP16)
        wv = np.concatenate([w_v_sw[:, h, :], w_v_g[:, h, :]], axis=1).astype(NP16)
        bq = np.concatenate([b_q_sw[h], b_q_g[h]]).reshape(2 * HD, 1).astype(np.float32)
        wo = np.asarray(w_out[h], NP16)
        in_maps.append({
            "xqT": xqT, "xkvT": xkvT,
            "wq": wq, "wk": wk, "wv": wv, "bq": bq,
            "wo": wo, "ml": ml, "mr": mr, "mlg": mlg,
        })

    res = run_bass_kernel_spmd(nc, in_maps, list(range(N_CORES)),
                               trace=_trace, tmpdir=_tmpdir)
    partial = np.stack([res.results[h]["out"] for h in range(N_CORES)])
    out = partial.astype(np.float32).sum(axis=0)

    # v-bias correction: softmax weights sum to 1, so a v bias adds
    # (b_v @ w_out) summed over heads -- a constant row per branch.
    b_base = np.asarray(b_out, np.float32)
    b_sw = b_base + np.einsum("hd,hdf->f", np.asarray(b_v_sw, np.float32),
                              np.asarray(w_out, np.float32))
    b_g = b_base + np.einsum("hd,hdf->f", np.asarray(b_v_g, np.float32),
                             np.asarray(w_out, np.float32))
    out += b_sw[None, :]
    if ng > 0:
        out[:ng] += (b_g - b_sw)[None, :]
    if _trace:
        kernel._last_results = res
    return out[None].astype(np.float32)


# revision 14
# speedup vs baseline: 120.6235x; 1.2187x over previous
"""Longformer attention (B=1, S=4096, D=512, H=8, HD=64, window=512, nglobal=64)
on 8 Trainium2 NeuronCores, head-parallel (core c computes head c).

v4 design (bf16 operands, fp32 PSUM accumulation):
  - Host pre-transposes inputs to xT [512, 4096] bf16.
  - Sliding-window q/k are projected with ROW-DUPLICATED weights
    ([w_sw | w_sw]) so the attention logit matmuls contract over the full
    128 partitions ([q;q].[k;k] = 2 q.k, compensated in the exp scale).
    This keeps the PE Hardware Activity Monitor's busy detector fed --
    half-array (K=64) matmul streams never unthrottle the PE clock from
    1.2 to 2.4 GHz (measured: phase C ran 72us continuously busy at
    K=4/8 with 64-row contractions).
  - Global-branch q_g (first 128 tokens) and k_g (all tokens) projected
    in separate small passes.
  - k/v biases eliminated mathematically (k bias shifts all logits of a
    query equally -> softmax no-op; v bias folds into a host-side output
    constant since softmax weights sum to 1).  q bias applied on ACT
    during evacuation when nonzero.
  - v produced directly in natural [s, d] layout; ones column appended
    (row-sum trick gives softmax denominators in row 64 of the AV psum).
  - Phase C is a 3-stage software pipeline: logits+exp+masks(t) //
    AV+cast(t-1) // denominators+out-proj+normalize+store(t-2).
  - Normalization folded into the PSUM evacuation as per-partition scale.
  - Host sums the 8 bf16 per-head partial outputs and adds bias terms.
"""
import os
import sys
import functools

for _p in ("/opt/trn_rl_repo",):
    if os.path.isdir(_p) and _p not in sys.path:
        sys.path.insert(0, _p)

import numpy as np

import concourse.bass as bass
import concourse.tile as tile
from concourse import bacc, mybir
from concourse.bass_utils import run_bass_kernel_spmd

S = 4096
F = 512          # d_model
HD = 64          # head dim
H = 8
WIN = 512        # sliding window (left 256, right 256)
ST = 256         # query supertile
NST = S // ST    # 16
KT = 128         # key tile
NKT = S // KT    # 32
N_CORES = 8
F32 = mybir.dt.float32
F16 = mybir.dt.bfloat16  # fp16 matmul is pathologically slow on TRN2 HW
NP16 = mybir.dt.np(F16)
NQG = 128        # q_g columns kept (>= ng)


def _build_masks(ng: int):
    """Static 0/1 masks for the transposed [k=128, q=256] logit tiles.

    For supertile t and ktile j, delta = j - 2t and d = q - k =
    qq - kk + (-delta)*128 with qq in [0,256), kk in [0,128).
    Band keeps d in [-256, 255].
    delta=-2 -> keep qq <= kk - 1;   delta=-1 -> keep qq <= kk + 127
    delta=+2 -> keep qq >= kk;       delta=+3 -> keep qq >= kk + 128
    """
    kk = np.arange(KT)[:, None]
    qq = np.arange(ST)[None, :]
    m_m2 = (qq <= kk - 1).astype(NP16)
    m_m1 = (qq <= kk + 127).astype(NP16)
    m_p2 = (qq >= kk).astype(NP16)
    m_p3 = (qq >= kk + 128).astype(NP16)
    ml = np.concatenate([m_m2, m_m1], axis=1)            # [128, 512]
    mr = np.concatenate([m_p2, m_p3], axis=1)            # [128, 512]
    m_m2g = m_m2.copy()
    if ng > 0:
        m_m2g[:ng, :] = 1.0                              # global k rows kept
    mlg = np.concatenate([m_m2g, m_m1], axis=1)          # used at t=1
    return ml, mr, mlg


def _sw_tiles(t: int):
    """ktile range and mask placements for supertile t."""
    j0 = max(0, 2 * t - 2)
    j1 = min(NKT, 2 * t + 4)
    ml_present = 2 * t - 2 >= 0
    mr_present = 2 * t + 2 < j1
    mr_off = (2 * t + 2 - j0) * ST if mr_present else None
    return j0, j1, ml_present, mr_off


def _build_program(ng: int, use_bq: bool):
    """Build + compile the per-core bass program, specialized for ng leading
    global tokens (0 <= ng <= 128)."""
    nc = bacc.Bacc("TRN2", target_bir_lowering=False, debug=False,
                   num_devices=N_CORES)

    d = {}
    d["xqT"] = nc.dram_tensor("xqT", [F, S], F16, kind="ExternalInput").ap()
    d["xkvT"] = nc.dram_tensor("xkvT", [F, S], F16, kind="ExternalInput").ap()
    for w in ("wq", "wk", "wv"):        # wq/wk are [w_sw | w_sw] duplicated
        d[w] = nc.dram_tensor(w, [F, 2 * HD], F16, kind="ExternalInput").ap()
    d["wqg"] = nc.dram_tensor("wqg", [F, HD], F16, kind="ExternalInput").ap()
    d["wkg"] = nc.dram_tensor("wkg", [F, HD], F16, kind="ExternalInput").ap()
    d["bq"] = nc.dram_tensor("bq", [2 * HD, 1], F32, kind="ExternalInput").ap()
    d["bqg"] = nc.dram_tensor("bqg", [HD, 1], F32, kind="ExternalInput").ap()
    d["wo"] = nc.dram_tensor("wo", [HD, F], F16, kind="ExternalInput").ap()
    d["ml"] = nc.dram_tensor("ml", [KT, 2 * ST], F16, kind="ExternalInput").ap()
    d["mr"] = nc.dram_tensor("mr", [KT, 2 * ST], F16, kind="ExternalInput").ap()
    d["mlg"] = nc.dram_tensor("mlg", [KT, 2 * ST], F16, kind="ExternalInput").ap()
    out_ap = nc.dram_tensor("out", [S, F], F16, kind="ExternalOutput").ap()

    SC = 512            # projection s-chunk (one psum bank)
    NSC = S // SC       # 8
    FT = F // 128       # 4 f-chunks
    Copy = mybir.ActivationFunctionType.Copy
    Exp = mybir.ActivationFunctionType.Exp

    with tile.TileContext(nc) as tc:
        with (
            tc.tile_pool(name="const", bufs=1) as constp,
            tc.tile_pool(name="big", bufs=1) as bigp,
        ):
            # ---- constants / persistent tensors ----
            wq_sb = constp.tile([128, FT, 128], F16, tag="wq")
            wk_sb = constp.tile([128, FT, 128], F16, tag="wk")
            wv_sb = constp.tile([128, FT, 128], F16, tag="wv")
            for wsb, wap in ((wq_sb, d["wq"]), (wk_sb, d["wk"]), (wv_sb, d["wv"])):
                nc.sync.dma_start(wsb[:], wap.rearrange("(c p) e -> p c e", p=128))
            wqg_sb = constp.tile([128, FT, HD], F16, tag="wqg")
            wkg_sb = constp.tile([128, FT, HD], F16, tag="wkg")
            nc.sync.dma_start(wqg_sb[:], d["wqg"].rearrange("(c p) e -> p c e", p=128))
            nc.sync.dma_start(wkg_sb[:], d["wkg"].rearrange("(c p) e -> p c e", p=128))
            bq_sb = constp.tile([128, 1], F32, tag="bq")
            bqg_sb = constp.tile([HD, 1], F32, tag="bqg")
            if use_bq:
                nc.sync.dma_start(bq_sb[:], d["bq"][:])
                nc.sync.dma_start(bqg_sb[:], d["bqg"][:])
            wo_sb = constp.tile([HD, F], F16, tag="wo")
            ml_sb = constp.tile([KT, 2 * ST], F16, tag="ml")
            mr_sb = constp.tile([KT, 2 * ST], F16, tag="mr")
            mlg_sb = constp.tile([KT, 2 * ST], F16, tag="mlg")
            one_sb = constp.tile([128, 1], F16, tag="one")
            nc.vector.memset(one_sb[:], 1.0)

            qT = bigp.tile([128, S], F16, tag="qT")      # [q_sw; q_sw]
            kT2 = bigp.tile([128, S], F16, tag="kT2")    # [k_sw; k_sw]
            kg_sb = bigp.tile([HD, S], F16, tag="kg")    # k_g
            qg_sb = bigp.tile([HD, NQG], F16, tag="qg")  # q_g (first NQG)
            vsw = bigp.tile([128, NKT, HD + 1], F16, tag="vsw")  # [s%128, kt, d|1]
            vg = bigp.tile([128, NKT, HD + 1], F16, tag="vg")
            nc.vector.memset(vsw[:, :, HD], 1.0)
            nc.vector.memset(vg[:, :, HD], 1.0)

            # ================= Phase A: projections =================
            with (
                tc.tile_pool(name="xin", bufs=3) as xinp,
                tc.tile_pool(name="pa", bufs=3, space="PSUM") as pap,
                tc.tile_pool(name="pg", bufs=2, space="PSUM") as pgp,
                tc.tile_pool(name="pv", bufs=2, space="PSUM") as pvp,
            ):
                for sc in range(NSC):
                    ss = sc * SC
                    xq_t = xinp.tile([128, FT, SC], F16, tag="xq")
                    xkv_t = xinp.tile([128, FT, SC], F16, tag="xkv")
                    nc.sync.dma_start(
                        xq_t[:], d["xqT"].rearrange("(c p) s -> p c s", p=128)[:, :, ss:ss + SC])
                    nc.sync.dma_start(
                        xkv_t[:], d["xkvT"].rearrange("(c p) s -> p c s", p=128)[:, :, ss:ss + SC])
                    if sc == 0:
                        # masks/wo are not needed until phase C; issue their
                        # loads after the first input chunk so they don't
                        # delay the first matmuls.
                        nc.sync.dma_start(ml_sb[:], d["ml"][:])
                        nc.sync.dma_start(mr_sb[:], d["mr"][:])
                        if ng > 0:
                            nc.sync.dma_start(mlg_sb[:], d["mlg"][:])
                        nc.sync.dma_start(wo_sb[:], d["wo"][:])

                    pq = pap.tile([128, SC], F32, tag="pa")
                    for ft in range(FT):
                        nc.tensor.matmul(pq[:], wq_sb[:, ft, :],
                                         xq_t[:, ft, :],
                                         start=(ft == 0), stop=(ft == FT - 1))
                    if use_bq:
                        nc.scalar.activation(qT[:, ss:ss + SC], pq[:], Copy,
                                             bias=bq_sb[:, 0:1])
                    else:
                        nc.scalar.activation(qT[:, ss:ss + SC], pq[:], Copy)

                    pk = pap.tile([128, SC], F32, tag="pa")
                    for ft in range(FT):
                        nc.tensor.matmul(pk[:], wk_sb[:, ft, :],
                                         xkv_t[:, ft, :],
                                         start=(ft == 0), stop=(ft == FT - 1))
                    # k bias shifts logits by a per-query constant ->
                    # softmax-invariant; skipped exactly.
                    nc.scalar.activation(kT2[:, ss:ss + SC], pk[:], Copy)

                    if ng > 0:
                        pkg = pgp.tile([HD, SC], F32, tag="pg")
                        for ft in range(FT):
                            nc.tensor.matmul(pkg[:], wkg_sb[:, ft, :],
                                             xkv_t[:, ft, :],
                                             start=(ft == 0), stop=(ft == FT - 1))
                        nc.scalar.activation(kg_sb[:, ss:ss + SC], pkg[:], Copy)
                        if sc == 0:
                            pqg = pgp.tile([HD, NQG], F32, tag="pg")
                            for ft in range(FT):
                                nc.tensor.matmul(pqg[:], wqg_sb[:, ft, :],
                                                 xq_t[:, ft, 0:NQG],
                                                 start=(ft == 0), stop=(ft == FT - 1))
                            if use_bq:
                                nc.scalar.activation(qg_sb[:], pqg[:], Copy,
                                                     bias=bqg_sb[:, 0:1])
                            else:
                                nc.scalar.activation(qg_sb[:], pqg[:], Copy)

                    # v in natural [s, d] layout: lhsT = x s-block
                    pv = pvp.tile([128, 4, 128], F32, tag="pv")
                    for sb in range(4):
                        sb0 = sb * 128
                        for ft in range(FT):
                            nc.tensor.matmul(pv[:, sb, :],
                                             xkv_t[:, ft, sb0:sb0 + 128],
                                             wv_sb[:, ft, :],
                                             start=(ft == 0), stop=(ft == FT - 1))
                    kt0 = ss // 128
                    nc.vector.tensor_copy(vsw[:, kt0:kt0 + 4, 0:HD],
                                          pv[:, :, 0:HD])
                    nc.vector.tensor_copy(vg[:, kt0:kt0 + 4, 0:HD],
                                          pv[:, :, HD:2 * HD])

            # ================= Phase B: global attention (rows < ng) ============
            if ng > 0:
                with (
                    tc.tile_pool(name="eg", bufs=1) as egp,
                    tc.tile_pool(name="gx", bufs=1) as gxp,
                    tc.tile_pool(name="pb", bufs=2, space="PSUM") as pbp,
                    tc.tile_pool(name="pbs", bufs=1, space="PSUM") as pbsp,
                    tc.tile_pool(name="pbx", bufs=1, space="PSUM") as pbxp,
                    tc.tile_pool(name="pbo", bufs=1, space="PSUM") as pbop,
                ):
                    GG = 512 // ng          # ktiles per psum bank
                    NGRP = NKT // GG
                    eg = egp.tile([128, NKT, ng], F16, tag="eg")
                    pxg = pbxp.tile([HD + 1, ng], F32, tag="xg")
                    # pipeline: logits+exp of group g overlap AV of group g-1
                    for grp in range(NGRP + 1):
                        if grp < NGRP:
                            plg = pbp.tile([128, GG * ng], F32, tag="lg")
                            for j in range(GG):
                                kt = grp * GG + j
                                nc.tensor.matmul(plg[:, j * ng:(j + 1) * ng],
                                                 kg_sb[:, kt * KT:(kt + 1) * KT],
                                                 qg_sb[:, 0:ng],
                                                 start=True, stop=True)
                            nc.scalar.activation(
                                eg[:, grp * GG:(grp + 1) * GG, :], plg[:], Exp,
                                scale=0.125)
                        if grp >= 1:
                            for j in range(GG):
                                kt = (grp - 1) * GG + j
                                nc.tensor.matmul(pxg[:], vg[:, kt, :],
                                                 eg[:, kt, :],
                                                 start=(kt == 0),
                                                 stop=(kt == NKT - 1))
                    xgT = gxp.tile([HD + 1, ng], F16, tag="xgT")
                    nc.vector.tensor_copy(xgT[:], pxg[:])
                    psg = pbsp.tile([ng, 1], F32, tag="sg")
                    nc.tensor.matmul(psg[:], xgT[HD:HD + 1, 0:ng],
                                     one_sb[HD:HD + 1, 0:1],
                                     start=True, stop=True)
                    rg = gxp.tile([ng, 1], F32, tag="rg")
                    nc.vector.reciprocal(rg[:], psg[:])
                    pog = pbop.tile([ng, F], F32, tag="og")
                    nc.tensor.matmul(pog[:], xgT[0:HD, 0:ng], wo_sb[:],
                                     start=True, stop=True)
                    og = gxp.tile([ng, F], F16, tag="og_sb")
                    nc.scalar.activation(og[:], pog[:], Copy, scale=rg[:, 0:1])
                    nc.sync.dma_start(out_ap[0:ng, :], og[:])

            # ================= Phase C: sliding-window attention ================
            # 3-stage software pipeline: stage1(t) = logits+exp+masks,
            # stage2(t) = AV + psum->sbuf cast, stage3(t) = denominators +
            # reciprocal + out-proj + normalize-evacuate + store.
            # Logit contraction is over 128 duplicated rows -> 2 q.k,
            # compensated by exp scale 1/16.
            with (
                tc.tile_pool(name="E", bufs=4) as ep,
                tc.tile_pool(name="xt", bufs=3) as xtp,
                tc.tile_pool(name="osb", bufs=2) as osbp,
                tc.tile_pool(name="rc", bufs=2) as rcp,
                tc.tile_pool(name="pL", bufs=3, space="PSUM") as pLp,
                tc.tile_pool(name="pX", bufs=2, space="PSUM") as pXp,
                tc.tile_pool(name="pS", bufs=1, space="PSUM") as pSp,
                tc.tile_pool(name="pO", bufs=1, space="PSUM") as pOp,
            ):
                Es = {}
                Egs = {}
                xts = {}

                def stage1(t):
                    qs = t * ST
                    j0, j1, ml_present, mr_off = _sw_tiles(t)
                    nkt = j1 - j0
                    has_g = ng > 0 and j0 > 0
                    E = ep.tile([128, 6 * ST], F16, tag="E")
                    Es[t] = E
                    # global-key prepend first: its small exp frees the pL
                    # slot early, keeping the pool at bufs=3 stall-free
                    if has_g:
                        plg2 = pLp.tile([ng, ST], F32, tag="L")
                        nc.tensor.matmul(plg2[:], kT2[:, 0:ng],
                                         qT[:, qs:qs + ST],
                                         start=True, stop=True)
                        Eg2 = ep.tile([ng, ST], F16, tag="Eg")
                        nc.scalar.activation(Eg2[:], plg2[:], Exp, scale=0.0625)
                        Egs[t] = Eg2
                    # 2-ktile groups: one psum bank each
                    for a in range(0, nkt, 2):
                        b = min(a + 2, nkt)
                        pl = pLp.tile([128, (b - a) * ST], F32, tag="L")
                        for s in range(b - a):
                            j = j0 + a + s
                            nc.tensor.matmul(pl[:, s * ST:(s + 1) * ST],
                                             kT2[:, j * KT:(j + 1) * KT],
                                             qT[:, qs:qs + ST],
                                             start=True, stop=True)
                        nc.scalar.activation(E[:, a * ST:b * ST], pl[:], Exp,
                                             scale=0.0625)
                    # masks (ML on gpsimd, MR on vector to balance engines)
                    if ml_present:
                        msk = mlg_sb if (t == 1 and ng > 0) else ml_sb
                        nc.gpsimd.tensor_mul(E[:, 0:2 * ST], E[:, 0:2 * ST],
                                             msk[:])
                    if mr_off is not None:
                        nc.vector.tensor_mul(E[:, mr_off:mr_off + 2 * ST],
                                             E[:, mr_off:mr_off + 2 * ST],
                                             mr_sb[:])

                def stage2(t):
                    j0, j1, _, _ = _sw_tiles(t)
                    nkt = j1 - j0
                    has_g = ng > 0 and j0 > 0
                    E = Es.pop(t)
                    # AV: xT' = [v|1].T @ expw.T -> [65, 256], sums in row 64
                    px = pXp.tile([HD + 1, ST], F32, tag="X")
                    for s in range(nkt):
                        j = j0 + s
                        nc.tensor.matmul(px[:], vsw[:, j, :],
                                         E[:, s * ST:(s + 1) * ST],
                                         start=(s == 0),
                                         stop=(s == nkt - 1 and not has_g))
                    if has_g:
                        nc.tensor.matmul(px[:], vsw[0:ng, 0, :], Egs.pop(t),
                                         start=False, stop=True)
                    xT = xtp.tile([HD + 1, ST], F16, tag="xT")
                    nc.vector.tensor_copy(xT[:], px[:])
                    xts[t] = xT

                def stage3(t):
                    xT = xts.pop(t)
                    # denominators (row 64) -> per-partition columns
                    ps = pSp.tile([128, 2], F32, tag="S")
                    for hf in range(2):
                        nc.tensor.matmul(ps[:, hf:hf + 1],
                                         xT[HD:HD + 1, hf * 128:(hf + 1) * 128],
                                         one_sb[HD:HD + 1, 0:1],
                                         start=True, stop=True)
                    rc = rcp.tile([128, 2], F32, tag="rc")
                    nc.vector.reciprocal(rc[:], ps[:])
                    po = pOp.tile([128, 2, F], F32, tag="O")
                    for hf in range(2):
                        nc.tensor.matmul(po[:, hf, :],
                                         xT[0:HD, hf * 128:(hf + 1) * 128],
                                         wo_sb[:], start=True, stop=True)
                    osb = osbp.tile([128, 2, F], F16, tag="osb")
                    # normalization folded into the evacuation copies
                    nc.vector.tensor_scalar_mul(osb[:, 0, :], po[:, 0, :],
                                                rc[:, 0:1])
                    nc.vector.tensor_scalar_mul(osb[:, 1, :], po[:, 1, :],
                                                rc[:, 1:2])
                    if t == 0 and ng > 0:
                        nc.sync.dma_start(out_ap[ng:128, :], osb[ng:128, 0, :])
                        nc.sync.dma_start(out_ap[128:256, :], osb[:, 1, :])
                    else:
                        nc.sync.dma_start(
                            out_ap.rearrange("(a p) f -> p a f", p=128)
                            [:, 2 * t:2 * t + 2, :], osb[:])

                for step in range(NST + 2):
                    if step < NST:
                        stage1(step)
                    if 1 <= step <= NST:
                        stage2(step - 1)
                    if step >= 2:
                        stage3(step - 2)

    nc.compile()
    return nc


@functools.lru_cache(maxsize=4)
def _get_program(ng: int, use_bq: bool):
    return _build_program(ng, use_bq)


def kernel(inputs_q, inputs_kv, global_mask,
           w_q_sw, b_q_sw, w_k_sw, b_k_sw, w_v_sw, b_v_sw,
           w_q_g, b_q_g, w_k_g, b_k_g, w_v_g, b_v_g,
           w_out, b_out,
           _trace=False, _tmpdir=None):
    gm = np.asarray(global_mask[0]).astype(bool)
    ng = int(gm.sum())
    assert gm[:ng].all() and not gm[ng:].any(), "global_mask must be a prefix mask"
    assert ng <= NQG, "kernel specialized for ng <= 128"
    assert ng == 0 or 512 % ng == 0, "kernel requires ng dividing 512"

    xqT = np.ascontiguousarray(np.asarray(inputs_q[0], np.float32).T).astype(NP16)
    xkvT = np.ascontiguousarray(np.asarray(inputs_kv[0], np.float32).T).astype(NP16)
    ml, mr, mlg = _build_masks(ng)

    use_bq = bool(np.any(b_q_sw) or np.any(b_q_g))
    nc = _get_program(ng, use_bq)

    in_maps = []
    for h in range(N_CORES):
        wq = np.concatenate([w_q_sw[:, h, :]] * 2, axis=1).astype(NP16)
        wk = np.concatenate([w_k_sw[:, h, :]] * 2, axis=1).astype(NP16)
        wv = np.concatenate([w_v_sw[:, h, :], w_v_g[:, h, :]], axis=1).astype(NP16)
        wqg = np.ascontiguousarray(w_q_g[:, h, :]).astype(NP16)
        wkg = np.ascontiguousarray(w_k_g[:, h, :]).astype(NP16)
        bq = np.concatenate([b_q_sw[h]] * 2).reshape(2 * HD, 1).astype(np.float32)
        bqg = np.asarray(b_q_g[h]).reshape(HD, 1).astype(np.float32)
        wo = np.asarray(w_out[h], NP16)
        in_maps.append({
            "xqT": xqT, "xkvT": xkvT,
            "wq": wq, "wk": wk, "wv": wv, "wqg": wqg, "wkg": wkg,
            "bq": bq, "bqg": bqg,
            "wo": wo, "ml": ml, "mr": mr, "mlg": mlg,
        })

    res = run_bass_kernel_spmd(nc, in_maps, list(range(N_CORES)),
                               trace=_trace, tmpdir=_tmpdir)
    partial = np.stack([res.results[h]["out"] for h in range(N_CORES)])
    out = partial.astype(np.float32).sum(axis=0)

    # v-bias correction: softmax weights sum to 1, so a v bias adds
    # (b_v @ w_out) summed over heads -- a constant row per branch.
    b_base = np.asarray(b_out, np.float32)
    b_sw = b_base + np.einsum("hd,hdf->f", np.asarray(b_v_sw, np.float32),
                              np.asarray(w_out, np.float32))
    b_g = b_base + np.einsum("hd,hdf->f", np.asarray(b_v_g, np.float32),
                             np.asarray(w_out, np.float32))
    out += b_sw[None, :]
    if ng > 0:
        out[:ng] += (b_g - b_sw)[None, :]
    if _trace:
        kernel._last_results = res
    return out[None].astype(np.float32)


# revision 16
# speedup vs baseline: 123.7937x; 1.0263x over previous
"""Longformer attention (B=1, S=4096, D=512, H=8, HD=64, window=512, nglobal=64)
on 8 Trainium2 NeuronCores, head-parallel (core c computes head c).

v5 design (bf16 operands, fp32 PSUM accumulation):
  - Host pre-transposes inputs to xT [512, 4096] bf16.
  - qT = [q_sw; ZEROS]: attention logit matmuls contract over the full
    128 partitions (the zero rows annihilate the k_g half of
    kT = [k_sw; k_g], so logits are exact).  Full-row contraction keeps
    the PE Hardware Activity Monitor's busy detector fed -- half-array
    (K=64) matmul streams never unthrottle the PE clock from 1.2 to
    2.4 GHz (measured: 72us continuously busy yet stuck at K=4/8).
  - q_g (for the ng<=128 global queries) projected into partitions
    64:128 via a col-group matmul so phase B operands share base
    partitions with kT's k_g rows.
  - k/v biases eliminated mathematically (k bias shifts all logits of a
    query equally -> softmax no-op; v bias folds into a host-side output
    constant since softmax weights sum to 1).  q bias applied during
    evacuation when nonzero.
  - v produced directly in natural [s, d] layout; ones column appended
    (row-sum trick gives softmax denominators in row 64 of the AV psum).
  - Phase C is a 3-stage software pipeline: logits+exp+masks(t) //
    AV+cast(t-1) // denominators+out-proj+normalize+store(t-2).
    Exp batched in 3-ktile groups (two ACT calls per supertile).
  - Global attention (phase B) is scheduled into phase C's pipeline
    drain steps, reusing phase C's PSUM pools.
  - Normalization folded into the PSUM evacuation as per-partition scale.
  - Host sums the 8 bf16 per-head partial outputs and adds bias terms.
"""
import os
import sys
import functools

for _p in ("/opt/trn_rl_repo",):
    if os.path.isdir(_p) and _p not in sys.path:
        sys.path.insert(0, _p)

import numpy as np

import concourse.bass as bass
import concourse.tile as tile
from concourse import bacc, mybir
from concourse.bass_utils import run_bass_kernel_spmd

S = 4096
F = 512          # d_model
HD = 64          # head dim
H = 8
WIN = 512        # sliding window (left 256, right 256)
ST = 256         # query supertile
NST = S // ST    # 16
KT = 128         # key tile
NKT = S // KT    # 32
N_CORES = 8
F32 = mybir.dt.float32
F16 = mybir.dt.bfloat16  # fp16 matmul is pathologically slow on TRN2 HW
NP16 = mybir.dt.np(F16)
NQG = 128        # q_g columns kept (>= ng)


def _build_masks(ng: int):
    """Static 0/1 masks for the transposed [k=128, q=256] logit tiles.

    For supertile t and ktile j, delta = j - 2t and d = q - k =
    qq - kk + (-delta)*128 with qq in [0,256), kk in [0,128).
    Band keeps d in [-256, 255].
    delta=-2 -> keep qq <= kk - 1;   delta=-1 -> keep qq <= kk + 127
    delta=+2 -> keep qq >= kk;       delta=+3 -> keep qq >= kk + 128
    """
    kk = np.arange(KT)[:, None]
    qq = np.arange(ST)[None, :]
    m_m2 = (qq <= kk - 1).astype(NP16)
    m_m1 = (qq <= kk + 127).astype(NP16)
    m_p2 = (qq >= kk).astype(NP16)
    m_p3 = (qq >= kk + 128).astype(NP16)
    ml = np.concatenate([m_m2, m_m1], axis=1)            # [128, 512]
    mr = np.concatenate([m_p2, m_p3], axis=1)            # [128, 512]
    m_m2g = m_m2.copy()
    if ng > 0:
        m_m2g[:ng, :] = 1.0                              # global k rows kept
    mlg = np.concatenate([m_m2g, m_m1], axis=1)          # used at t=1
    return ml, mr, mlg


def _sw_tiles(t: int):
    """ktile range and mask placements for supertile t."""
    j0 = max(0, 2 * t - 2)
    j1 = min(NKT, 2 * t + 4)
    ml_present = 2 * t - 2 >= 0
    mr_present = 2 * t + 2 < j1
    mr_off = (2 * t + 2 - j0) * ST if mr_present else None
    return j0, j1, ml_present, mr_off


def _build_program(ng: int, use_bq: bool):
    """Build + compile the per-core bass program, specialized for ng leading
    global tokens (0 <= ng <= 128)."""
    nc = bacc.Bacc("TRN2", target_bir_lowering=False, debug=False,
                   num_devices=N_CORES)

    d = {}
    d["xqT"] = nc.dram_tensor("xqT", [F, S], F16, kind="ExternalInput").ap()
    d["xkvT"] = nc.dram_tensor("xkvT", [F, S], F16, kind="ExternalInput").ap()
    for w in ("wq", "wk", "wv"):  # wq = [w_q_sw | 0], wk = [w_k_sw | w_k_g]
        d[w] = nc.dram_tensor(w, [F, 2 * HD], F16, kind="ExternalInput").ap()
    d["wqg"] = nc.dram_tensor("wqg", [F, HD], F16, kind="ExternalInput").ap()
    d["bq"] = nc.dram_tensor("bq", [2 * HD, 1], F32, kind="ExternalInput").ap()
    d["bqg"] = nc.dram_tensor("bqg", [HD, 1], F32, kind="ExternalInput").ap()
    d["wo"] = nc.dram_tensor("wo", [HD, F], F16, kind="ExternalInput").ap()
    d["ml"] = nc.dram_tensor("ml", [KT, 2 * ST], F16, kind="ExternalInput").ap()
    d["mr"] = nc.dram_tensor("mr", [KT, 2 * ST], F16, kind="ExternalInput").ap()
    d["mlg"] = nc.dram_tensor("mlg", [KT, 2 * ST], F16, kind="ExternalInput").ap()
    out_ap = nc.dram_tensor("out", [S, F], F16, kind="ExternalOutput").ap()

    SC = 512            # projection s-chunk (one psum bank)
    NSC = S // SC       # 8
    FT = F // 128       # 4 f-chunks
    Copy = mybir.ActivationFunctionType.Copy
    Exp = mybir.ActivationFunctionType.Exp
    xq_r = d["xqT"].rearrange("(c p) s -> p c s", p=128)
    xkv_r = d["xkvT"].rearrange("(c p) s -> p c s", p=128)

    with tile.TileContext(nc) as tc:
        with (
            tc.tile_pool(name="const", bufs=1) as constp,
            tc.tile_pool(name="big", bufs=1) as bigp,
        ):
            # ---- constants / persistent tensors ----
            wq_sb = constp.tile([128, FT, 128], F16, tag="wq")
            wk_sb = constp.tile([128, FT, 128], F16, tag="wk")
            wv_sb = constp.tile([128, FT, 128], F16, tag="wv")
            for wsb, wap in ((wq_sb, d["wq"]), (wk_sb, d["wk"]), (wv_sb, d["wv"])):
                nc.sync.dma_start(wsb[:], wap.rearrange("(c p) e -> p c e", p=128))
            wqg_sb = constp.tile([128, FT, HD], F16, tag="wqg")
            if ng > 0:
                nc.sync.dma_start(wqg_sb[:], d["wqg"].rearrange("(c p) e -> p c e", p=128))
            bq_sb = constp.tile([128, 1], F32, tag="bq")
            bqg_sb = constp.tile([128, 1], F32, tag="bqg")
            if use_bq:
                nc.sync.dma_start(bq_sb[:], d["bq"][:])
                if ng > 0:
                    nc.sync.dma_start(bqg_sb[64:128, :], d["bqg"][:])
            wo_sb = constp.tile([HD, F], F16, tag="wo")
            ml_sb = constp.tile([KT, 2 * ST], F16, tag="ml")
            mr_sb = constp.tile([KT, 2 * ST], F16, tag="mr")
            mlg_sb = constp.tile([KT, 2 * ST], F16, tag="mlg")
            one_sb = constp.tile([128, 1], F16, tag="one")
            nc.vector.memset(one_sb[:], 1.0)

            qT = bigp.tile([128, S], F16, tag="qT")      # [q_sw; 0]
            kT = bigp.tile([128, S], F16, tag="kT")      # [k_sw; k_g]
            qg_sb = bigp.tile([128, NQG], F16, tag="qg")  # q_g in rows 64:128
            vsw = bigp.tile([128, NKT, HD + 1], F16, tag="vsw")  # [s%128, kt, d|1]
            vg = bigp.tile([128, NKT, HD + 1], F16, tag="vg")
            nc.vector.memset(vsw[:, :, HD], 1.0)
            nc.vector.memset(vg[:, :, HD], 1.0)

            # ================= Phase A: projections =================
            with (
                tc.tile_pool(name="xin", bufs=3) as xinp,
                tc.tile_pool(name="pa", bufs=3, space="PSUM") as pap,
                tc.tile_pool(name="pg", bufs=1, space="PSUM") as pgp,
                tc.tile_pool(name="pv", bufs=2, space="PSUM") as pvp,
            ):
                for sc in range(NSC):
                    ss = sc * SC
                    xq_t = xinp.tile([128, FT, SC], F16, tag="xq")
                    xkv_t = xinp.tile([128, FT, SC], F16, tag="xkv")
                    if sc == 0:
                        # per-f-chunk pieces so the first matmul can start
                        # as soon as its rows land
                        for ft in range(FT):
                            nc.sync.dma_start(xq_t[:, ft, :],
                                              xq_r[:, ft, ss:ss + SC])
                            nc.sync.dma_start(xkv_t[:, ft, :],
                                              xkv_r[:, ft, ss:ss + SC])
                        # masks/wo are not needed until phase C; issue their
                        # loads after the first input chunk
                        nc.sync.dma_start(ml_sb[:], d["ml"][:])
                        nc.sync.dma_start(mr_sb[:], d["mr"][:])
                        if ng > 0:
                            nc.sync.dma_start(mlg_sb[:], d["mlg"][:])
                        nc.sync.dma_start(wo_sb[:], d["wo"][:])
                    else:
                        nc.sync.dma_start(xq_t[:], xq_r[:, :, ss:ss + SC])
                        nc.sync.dma_start(xkv_t[:], xkv_r[:, :, ss:ss + SC])

                    pq = pap.tile([128, SC], F32, tag="pa")
                    for ft in range(FT):
                        nc.tensor.matmul(pq[:], wq_sb[:, ft, :],
                                         xq_t[:, ft, :],
                                         start=(ft == 0), stop=(ft == FT - 1))
                    if use_bq:
                        nc.scalar.activation(qT[:, ss:ss + SC], pq[:], Copy,
                                             bias=bq_sb[:, 0:1])
                    else:
                        nc.scalar.activation(qT[:, ss:ss + SC], pq[:], Copy)

                    pk = pap.tile([128, SC], F32, tag="pa")
                    for ft in range(FT):
                        nc.tensor.matmul(pk[:], wk_sb[:, ft, :],
                                         xkv_t[:, ft, :],
                                         start=(ft == 0), stop=(ft == FT - 1))
                    # k bias shifts logits by a per-query constant ->
                    # softmax-invariant; skipped exactly.
                    nc.scalar.activation(kT[:, ss:ss + SC], pk[:], Copy)

                    if ng > 0 and sc == 0:
                        # q_g into partitions 64:128 (col-group matmul)
                        pqg = pgp.tile([128, NQG], F32, tag="pg")
                        for ft in range(FT):
                            nc.tensor.matmul(pqg[64:128, :], wqg_sb[:, ft, :],
                                             xq_t[:, ft, 0:NQG],
                                             start=(ft == 0), stop=(ft == FT - 1))
                        if use_bq:
                            nc.scalar.activation(qg_sb[64:128, :],
                                                 pqg[64:128, :], Copy,
                                                 bias=bqg_sb[64:128, 0:1])
                        else:
                            nc.scalar.activation(qg_sb[64:128, :],
                                                 pqg[64:128, :], Copy)

                    # v in natural [s, d] layout: lhsT = x s-block
                    pv = pvp.tile([128, 4, 128], F32, tag="pv")
                    for sb in range(4):
                        sb0 = sb * 128
                        for ft in range(FT):
                            nc.tensor.matmul(pv[:, sb, :],
                                             xkv_t[:, ft, sb0:sb0 + 128],
                                             wv_sb[:, ft, :],
                                             start=(ft == 0), stop=(ft == FT - 1))
                    kt0 = ss // 128
                    nc.vector.tensor_copy(vsw[:, kt0:kt0 + 4, 0:HD],
                                          pv[:, :, 0:HD])
                    nc.vector.tensor_copy(vg[:, kt0:kt0 + 4, 0:HD],
                                          pv[:, :, HD:2 * HD])

            # ====== Phases B + C: global + sliding-window attention ======
            # Phase C is a 3-stage software pipeline; phase B's work is
            # injected into the pipeline drain steps, reusing C's psum pools.
            GG = 512 // ng if ng > 0 else 1   # B ktiles per psum bank
            NGRP = NKT // GG
            with (
                tc.tile_pool(name="E", bufs=4) as ep,
                tc.tile_pool(name="xt", bufs=3) as xtp,
                tc.tile_pool(name="osb", bufs=2) as osbp,
                tc.tile_pool(name="rc", bufs=2) as rcp,
                tc.tile_pool(name="gx", bufs=1) as gxp,
                tc.tile_pool(name="pL", bufs=2, space="PSUM") as pLp,
                tc.tile_pool(name="pX", bufs=2, space="PSUM") as pXp,
                tc.tile_pool(name="pO", bufs=1, space="PSUM") as pOp,
            ):
                Es = {}
                Egs = {}
                xts = {}
                eg = gxp.tile([128, NKT, max(ng, 1)], F16, tag="eg")
                Bst = {"pxg": None}

                def stage1(t):
                    qs = t * ST
                    j0, j1, ml_present, mr_off = _sw_tiles(t)
                    nkt = j1 - j0
                    has_g = ng > 0 and j0 > 0
                    E = ep.tile([128, 6 * ST], F16, tag="E")
                    Es[t] = E
                    # global-key prepend first: its small exp frees the pL
                    # slot early, keeping the pool stall-free at bufs=2
                    if has_g:
                        plg2 = pLp.tile([ng, ST], F32, tag="L")
                        nc.tensor.matmul(plg2[:], kT[:, 0:ng],
                                         qT[:, qs:qs + ST],
                                         start=True, stop=True)
                        Eg2 = ep.tile([ng, ST], F16, tag="Eg")
                        nc.scalar.activation(Eg2[:], plg2[:], Exp, scale=0.125)
                        Egs[t] = Eg2
                    # 3-ktile groups (two ACT calls per supertile)
                    for a in range(0, nkt, 3):
                        b = min(a + 3, nkt)
                        pl = pLp.tile([128, (b - a) * ST], F32, tag="L")
                        for s in range(b - a):
                            j = j0 + a + s
                            nc.tensor.matmul(pl[:, s * ST:(s + 1) * ST],
                                             kT[:, j * KT:(j + 1) * KT],
                                             qT[:, qs:qs + ST],
                                             start=True, stop=True)
                        nc.scalar.activation(E[:, a * ST:b * ST], pl[:], Exp,
                                             scale=0.125)
                    # masks (ML on gpsimd, MR on vector to balance engines)
                    if ml_present:
                        msk = mlg_sb if (t == 1 and ng > 0) else ml_sb
                        nc.gpsimd.tensor_mul(E[:, 0:2 * ST], E[:, 0:2 * ST],
                                             msk[:])
                    if mr_off is not None:
                        nc.vector.tensor_mul(E[:, mr_off:mr_off + 2 * ST],
                                             E[:, mr_off:mr_off + 2 * ST],
                                             mr_sb[:])

                def stage2(t):
                    j0, j1, _, _ = _sw_tiles(t)
                    nkt = j1 - j0
                    has_g = ng > 0 and j0 > 0
                    E = Es.pop(t)
                    # AV: xT' = [v|1].T @ expw.T -> [65, 256], sums in row 64
                    px = pXp.tile([HD + 1, ST], F32, tag="X")
                    for s in range(nkt):
                        j = j0 + s
                        nc.tensor.matmul(px[:], vsw[:, j, :],
                                         E[:, s * ST:(s + 1) * ST],
                                         start=(s == 0),
                                         stop=(s == nkt - 1 and not has_g))
                    if has_g:
                        nc.tensor.matmul(px[:], vsw[0:ng, 0, :], Egs.pop(t),
                                         start=False, stop=True)
                    xT = xtp.tile([HD + 1, ST], F16, tag="xT")
                    nc.vector.tensor_copy(xT[:], px[:])
                    xts[t] = xT

                def stage3(t):
                    xT = xts.pop(t)
                    # denominators (row 64) -> per-partition columns
                    ps = pXp.tile([128, 2], F32, tag="X")
                    for hf in range(2):
                        nc.tensor.matmul(ps[:, hf:hf + 1],
                                         xT[HD:HD + 1, hf * 128:(hf + 1) * 128],
                                         one_sb[HD:HD + 1, 0:1],
                                         start=True, stop=True)
                    rc = rcp.tile([128, 2], F32, tag="rc")
                    nc.vector.reciprocal(rc[:], ps[:])
                    po = pOp.tile([128, 2, F], F32, tag="O")
                    for hf in range(2):
                        nc.tensor.matmul(po[:, hf, :],
                                         xT[0:HD, hf * 128:(hf + 1) * 128],
                                         wo_sb[:], start=True, stop=True)
                    osb = osbp.tile([128, 2, F], F16, tag="osb")
                    # normalization folded into the evacuation copies
                    nc.vector.tensor_scalar_mul(osb[:, 0, :], po[:, 0, :],
                                                rc[:, 0:1])
                    nc.vector.tensor_scalar_mul(osb[:, 1, :], po[:, 1, :],
                                                rc[:, 1:2])
                    if t == 0 and ng > 0:
                        nc.sync.dma_start(out_ap[ng:128, :], osb[ng:128, 0, :])
                        nc.sync.dma_start(out_ap[128:256, :], osb[:, 1, :])
                    else:
                        nc.sync.dma_start(
                            out_ap.rearrange("(a p) f -> p a f", p=128)
                            [:, 2 * t:2 * t + 2, :], osb[:])

                def b_group(grp):
                    # global-branch logits+exp for ktile group grp, and AV
                    # for group grp-1 (pipelined)
                    if grp < NGRP:
                        plg = pLp.tile([128, GG * ng], F32, tag="L")
                        for j in range(GG):
                            kt = grp * GG + j
                            nc.tensor.matmul(plg[:, j * ng:(j + 1) * ng],
                                             kT[64:128, kt * KT:(kt + 1) * KT],
                                             qg_sb[64:128, 0:ng],
                                             start=True, stop=True)
                        nc.scalar.activation(
                            eg[:, grp * GG:(grp + 1) * GG, :], plg[:], Exp,
                            scale=0.125)
                    if grp >= 1:
                        if Bst["pxg"] is None:
                            Bst["pxg"] = pXp.tile([HD + 1, ng], F32, tag="X",
                                                  name="pxg")
                        pxg = Bst["pxg"]
                        for j in range(GG):
                            kt = (grp - 1) * GG + j
                            nc.tensor.matmul(pxg[:], vg[:, kt, :],
                                             eg[:, kt, :],
                                             start=(kt == 0),
                                             stop=(kt == NKT - 1))

                def b_finish():
                    pxg = Bst["pxg"]
                    xgT = gxp.tile([HD + 1, ng], F16, tag="xgT")
                    nc.vector.tensor_copy(xgT[:], pxg[:])
                    psg = pXp.tile([ng, 1], F32, tag="X")
                    nc.tensor.matmul(psg[:], xgT[HD:HD + 1, 0:ng],
                                     one_sb[HD:HD + 1, 0:1],
                                     start=True, stop=True)
                    rg = gxp.tile([ng, 1], F32, tag="rg")
                    nc.vector.reciprocal(rg[:], psg[:])
                    pog = pOp.tile([ng, F], F32, tag="O")
                    nc.tensor.matmul(pog[:], xgT[0:HD, 0:ng], wo_sb[:],
                                     start=True, stop=True)
                    og = gxp.tile([ng, F], F16, tag="og_sb")
                    nc.scalar.activation(og[:], pog[:], Copy, scale=rg[:, 0:1])
                    nc.sync.dma_start(out_ap[0:ng, :], og[:])

                # B group g is issued at step NST + g (the pipeline drain);
                # b_finish after the last B AV.
                NSTEP = max(NST + 2, NST + NGRP + 2) if ng > 0 else NST + 2
                for step in range(NSTEP):
                    if step < NST:
                        stage1(step)
                    if 1 <= step <= NST:
                        stage2(step - 1)
                    if step >= 2 and step - 2 < NST:
                        stage3(step - 2)
                    if ng > 0 and NST <= step <= NST + NGRP:
                        b_group(step - NST)
                        if step == NST + NGRP:
                            b_finish()

    nc.compile()
    return nc


@functools.lru_cache(maxsize=4)
def _get_program(ng: int, use_bq: bool):
    return _build_program(ng, use_bq)


def kernel(inputs_q, inputs_kv, global_mask,
           w_q_sw, b_q_sw, w_k_sw, b_k_sw, w_v_sw, b_v_sw,
           w_q_g, b_q_g, w_k_g, b_k_g, w_v_g, b_v_g,
           w_out, b_out,
           _trace=False, _tmpdir=None):
    gm = np.asarray(global_mask[0]).astype(bool)
    ng = int(gm.sum())
    assert gm[:ng].all() and not gm[ng:].any(), "global_mask must be a prefix mask"
    assert ng <= NQG, "kernel specialized for ng <= 128"
    assert ng == 0 or 512 % ng == 0, "kernel requires ng dividing 512"

    xqT = np.ascontiguousarray(np.asarray(inputs_q[0], np.float32).T).astype(NP16)
    xkvT = np.ascontiguousarray(np.asarray(inputs_kv[0], np.float32).T).astype(NP16)
    ml, mr, mlg = _build_masks(ng)

    use_bq = bool(np.any(b_q_sw) or np.any(b_q_g))
    nc = _get_program(ng, use_bq)

    zeros_hd = np.zeros((F, HD), np.float32)
    in_maps = []
    for h in range(N_CORES):
        wq = np.concatenate([w_q_sw[:, h, :], zeros_hd], axis=1).astype(NP16)
        wk = np.concatenate([w_k_sw[:, h, :], w_k_g[:, h, :]], axis=1).astype(NP16)
        wv = np.concatenate([w_v_sw[:, h, :], w_v_g[:, h, :]], axis=1).astype(NP16)
        wqg = np.ascontiguousarray(w_q_g[:, h, :]).astype(NP16)
        bq = np.concatenate([b_q_sw[h], np.zeros((HD,), np.float32)]
                            ).reshape(2 * HD, 1).astype(np.float32)
        bqg = np.asarray(b_q_g[h]).reshape(HD, 1).astype(np.float32)
        wo = np.asarray(w_out[h], NP16)
        in_maps.append({
            "xqT": xqT, "xkvT": xkvT,
            "wq": wq, "wk": wk, "wv": wv, "wqg": wqg,
            "bq": bq, "bqg": bqg,
            "wo": wo, "ml": ml, "mr": mr, "mlg": mlg,
        })

    res = run_bass_kernel_spmd(nc, in_maps, list(range(N_CORES)),
                               trace=_trace, tmpdir=_tmpdir)
    partial = np.stack([res.results[h]["out"] for h in range(N_CORES)])
    out = partial.astype(np.float32).sum(axis=0)

    # v-bias correction: softmax weights sum to 1, so a v bias adds
    # (b_v @ w_out) summed over heads -- a constant row per branch.
    b_base = np.asarray(b_out, np.float32)
    b_sw = b_base + np.einsum("hd,hdf->f", np.asarray(b_v_sw, np.float32),
                              np.asarray(w_out, np.float32))
    b_g = b_base + np.einsum("hd,hdf->f", np.asarray(b_v_g, np.float32),
                             np.asarray(w_out, np.float32))
    out += b_sw[None, :]
    if ng > 0:
        out[:ng] += (b_g - b_sw)[None, :]
    if _trace:
        kernel._last_results = res
    return out[None].astype(np.float32)


# revision 17
# speedup vs baseline: 128.4088x; 1.0373x over previous
"""Longformer attention (B=1, S=4096, D=512, H=8, HD=64, window=512, nglobal=64)
on 8 Trainium2 NeuronCores, head-parallel (core c computes head c).

v5 design (bf16 operands, fp32 PSUM accumulation):
  - Host pre-transposes inputs to xT [512, 4096] bf16.
  - qT = [q_sw; ZEROS]: attention logit matmuls contract over the full
    128 partitions (the zero rows annihilate the k_g half of
    kT = [k_sw; k_g], so logits are exact).  Full-row contraction keeps
    the PE Hardware Activity Monitor's busy detector fed -- half-array
    (K=64) matmul streams never unthrottle the PE clock from 1.2 to
    2.4 GHz (measured: 72us continuously busy yet stuck at K=4/8).
  - q_g (for the ng<=128 global queries) projected into partitions
    64:128 via a col-group matmul so phase B operands share base
    partitions with kT's k_g rows.
  - k/v biases eliminated mathematically (k bias shifts all logits of a
    query equally -> softmax no-op; v bias folds into a host-side output
    constant since softmax weights sum to 1).  q bias applied during
    evacuation when nonzero.
  - v produced directly in natural [s, d] layout; ones column appended
    (row-sum trick gives softmax denominators in row 64 of the AV psum).
  - Phase C is a 3-stage software pipeline: logits+exp+masks(t) //
    AV+cast(t-1) // denominators+out-proj+normalize+store(t-2).
    Exp batched in 3-ktile groups (two ACT calls per supertile).
  - Global attention (phase B) is scheduled into phase C's pipeline
    drain steps, reusing phase C's PSUM pools.
  - Normalization folded into the PSUM evacuation as per-partition scale.
  - Host sums the 8 bf16 per-head partial outputs and adds bias terms.
"""
import os
import sys
import functools

for _p in ("/opt/trn_rl_repo",):
    if os.path.isdir(_p) and _p not in sys.path:
        sys.path.insert(0, _p)

import numpy as np

import concourse.bass as bass
import concourse.tile as tile
from concourse import bacc, mybir
from concourse.bass_utils import run_bass_kernel_spmd

S = 4096
F = 512          # d_model
HD = 64          # head dim
H = 8
WIN = 512        # sliding window (left 256, right 256)
ST = 256         # query supertile
NST = S // ST    # 16
KT = 128         # key tile
NKT = S // KT    # 32
N_CORES = 8
F32 = mybir.dt.float32
F16 = mybir.dt.bfloat16  # fp16 matmul is pathologically slow on TRN2 HW
NP16 = mybir.dt.np(F16)
NQG = 128        # q_g columns kept (>= ng)


def _build_masks(ng: int):
    """Static 0/1 masks for the transposed [k=128, q=256] logit tiles.

    For supertile t and ktile j, delta = j - 2t and d = q - k =
    qq - kk + (-delta)*128 with qq in [0,256), kk in [0,128).
    Band keeps d in [-256, 255].
    delta=-2 -> keep qq <= kk - 1;   delta=-1 -> keep qq <= kk + 127
    delta=+2 -> keep qq >= kk;       delta=+3 -> keep qq >= kk + 128
    """
    kk = np.arange(KT)[:, None]
    qq = np.arange(ST)[None, :]
    m_m2 = (qq <= kk - 1).astype(NP16)
    m_m1 = (qq <= kk + 127).astype(NP16)
    m_p2 = (qq >= kk).astype(NP16)
    m_p3 = (qq >= kk + 128).astype(NP16)
    ml = np.concatenate([m_m2, m_m1], axis=1)            # [128, 512]
    mr = np.concatenate([m_p2, m_p3], axis=1)            # [128, 512]
    m_m2g = m_m2.copy()
    if ng > 0:
        m_m2g[:ng, :] = 1.0                              # global k rows kept
    mlg = np.concatenate([m_m2g, m_m1], axis=1)          # used at t=1
    return ml, mr, mlg


def _sw_tiles(t: int):
    """ktile range and mask placements for supertile t."""
    j0 = max(0, 2 * t - 2)
    j1 = min(NKT, 2 * t + 4)
    ml_present = 2 * t - 2 >= 0
    mr_present = 2 * t + 2 < j1
    mr_off = (2 * t + 2 - j0) * ST if mr_present else None
    return j0, j1, ml_present, mr_off


def _build_program(ng: int, use_bq: bool):
    """Build + compile the per-core bass program, specialized for ng leading
    global tokens (0 <= ng <= 128)."""
    nc = bacc.Bacc("TRN2", target_bir_lowering=False, debug=False,
                   num_devices=N_CORES)

    SC = 512            # projection s-chunk (one psum bank)
    NSC = S // SC       # 8
    FT = F // 128       # 4 f-chunks
    d = {}
    # inputs pre-permuted on host to [128, NSC, FT, SC] so every DMA
    # chunk is one contiguous 4KB run per partition
    d["xqT"] = nc.dram_tensor("xqT", [128, S * FT], F16, kind="ExternalInput").ap()
    d["xkvT"] = nc.dram_tensor("xkvT", [128, S * FT], F16, kind="ExternalInput").ap()
    for w in ("wq", "wk", "wv"):  # wq = [w_q_sw | 0], wk = [w_k_sw | w_k_g]
        d[w] = nc.dram_tensor(w, [F, 2 * HD], F16, kind="ExternalInput").ap()
    d["wqg"] = nc.dram_tensor("wqg", [F, HD], F16, kind="ExternalInput").ap()
    d["bq"] = nc.dram_tensor("bq", [2 * HD, 1], F32, kind="ExternalInput").ap()
    d["bqg"] = nc.dram_tensor("bqg", [HD, 1], F32, kind="ExternalInput").ap()
    d["wo"] = nc.dram_tensor("wo", [HD, F], F16, kind="ExternalInput").ap()
    d["ml"] = nc.dram_tensor("ml", [KT, 2 * ST], F16, kind="ExternalInput").ap()
    d["mr"] = nc.dram_tensor("mr", [KT, 2 * ST], F16, kind="ExternalInput").ap()
    d["mlg"] = nc.dram_tensor("mlg", [KT, 2 * ST], F16, kind="ExternalInput").ap()
    out_ap = nc.dram_tensor("out", [S, F], F16, kind="ExternalOutput").ap()

    Copy = mybir.ActivationFunctionType.Copy
    Exp = mybir.ActivationFunctionType.Exp
    CW = FT * SC        # per-chunk contiguous run per partition
    xq_r = d["xqT"]
    xkv_r = d["xkvT"]

    with tile.TileContext(nc) as tc:
        with (
            tc.tile_pool(name="const", bufs=1) as constp,
            tc.tile_pool(name="big", bufs=1) as bigp,
        ):
            # ---- constants / persistent tensors ----
            wq_sb = constp.tile([128, FT, 128], F16, tag="wq")
            wk_sb = constp.tile([128, FT, 128], F16, tag="wk")
            wv_sb = constp.tile([128, FT, 128], F16, tag="wv")
            for wsb, wap in ((wq_sb, d["wq"]), (wk_sb, d["wk"]), (wv_sb, d["wv"])):
                nc.sync.dma_start(wsb[:], wap.rearrange("(c p) e -> p c e", p=128))
            wqg_sb = constp.tile([128, FT, HD], F16, tag="wqg")
            if ng > 0:
                nc.sync.dma_start(wqg_sb[:], d["wqg"].rearrange("(c p) e -> p c e", p=128))
            bq_sb = constp.tile([128, 1], F32, tag="bq")
            bqg_sb = constp.tile([128, 1], F32, tag="bqg")
            if use_bq:
                nc.sync.dma_start(bq_sb[:], d["bq"][:])
                if ng > 0:
                    nc.sync.dma_start(bqg_sb[64:128, :], d["bqg"][:])
            wo_sb = constp.tile([HD, F], F16, tag="wo")
            ml_sb = constp.tile([KT, 2 * ST], F16, tag="ml")
            mr_sb = constp.tile([KT, 2 * ST], F16, tag="mr")
            mlg_sb = constp.tile([KT, 2 * ST], F16, tag="mlg")
            one_sb = constp.tile([128, 1], F16, tag="one")
            nc.vector.memset(one_sb[:], 1.0)

            qT = bigp.tile([128, S], F16, tag="qT")      # [q_sw; 0]
            kT = bigp.tile([128, S], F16, tag="kT")      # [k_sw; k_g]
            qg_sb = bigp.tile([128, NQG], F16, tag="qg")  # q_g in rows 64:128
            vsw = bigp.tile([128, NKT, HD + 1], F16, tag="vsw")  # [s%128, kt, d|1]
            vg = bigp.tile([128, NKT, HD + 1], F16, tag="vg")
            nc.vector.memset(vsw[:, :, HD], 1.0)
            nc.vector.memset(vg[:, :, HD], 1.0)

            # ================= Phase A: projections =================
            with (
                tc.tile_pool(name="xin", bufs=3) as xinp,
                tc.tile_pool(name="pa", bufs=3, space="PSUM") as pap,
                tc.tile_pool(name="pg", bufs=1, space="PSUM") as pgp,
                tc.tile_pool(name="pv", bufs=2, space="PSUM") as pvp,
            ):
                for sc in range(NSC):
                    ss = sc * SC
                    xq_t = xinp.tile([128, FT, SC], F16, tag="xq")
                    xkv_t = xinp.tile([128, FT, SC], F16, tag="xkv")
                    if sc == 0:
                        # per-f-chunk pieces so the first matmul can start
                        # as soon as its rows land
                        for ft in range(FT):
                            nc.sync.dma_start(
                                xq_t[:, ft, :],
                                xq_r[:, sc * CW + ft * SC:sc * CW + (ft + 1) * SC])
                            nc.sync.dma_start(
                                xkv_t[:, ft, :],
                                xkv_r[:, sc * CW + ft * SC:sc * CW + (ft + 1) * SC])
                        # masks/wo are not needed until phase C; issue their
                        # loads after the first input chunk
                        nc.sync.dma_start(ml_sb[:], d["ml"][:])
                        nc.sync.dma_start(mr_sb[:], d["mr"][:])
                        if ng > 0:
                            nc.sync.dma_start(mlg_sb[:], d["mlg"][:])
                        nc.sync.dma_start(wo_sb[:], d["wo"][:])
                    else:
                        nc.sync.dma_start(xq_t[:], xq_r[:, sc * CW:(sc + 1) * CW])
                        nc.sync.dma_start(xkv_t[:], xkv_r[:, sc * CW:(sc + 1) * CW])

                    pq = pap.tile([128, SC], F32, tag="pa")
                    for ft in range(FT):
                        nc.tensor.matmul(pq[:], wq_sb[:, ft, :],
                                         xq_t[:, ft, :],
                                         start=(ft == 0), stop=(ft == FT - 1))
                    if use_bq:
                        nc.scalar.activation(qT[:, ss:ss + SC], pq[:], Copy,
                                             bias=bq_sb[:, 0:1])
                    else:
                        nc.scalar.activation(qT[:, ss:ss + SC], pq[:], Copy)

                    pk = pap.tile([128, SC], F32, tag="pa")
                    for ft in range(FT):
                        nc.tensor.matmul(pk[:], wk_sb[:, ft, :],
                                         xkv_t[:, ft, :],
                                         start=(ft == 0), stop=(ft == FT - 1))
                    # k bias shifts logits by a per-query constant ->
                    # softmax-invariant; skipped exactly.
                    nc.scalar.activation(kT[:, ss:ss + SC], pk[:], Copy)

                    if ng > 0 and sc == 0:
                        # q_g into partitions 64:128 (col-group matmul)
                        pqg = pgp.tile([128, NQG], F32, tag="pg")
                        for ft in range(FT):
                            nc.tensor.matmul(pqg[64:128, :], wqg_sb[:, ft, :],
                                             xq_t[:, ft, 0:NQG],
                                             start=(ft == 0), stop=(ft == FT - 1))
                        if use_bq:
                            nc.scalar.activation(qg_sb[64:128, :],
                                                 pqg[64:128, :], Copy,
                                                 bias=bqg_sb[64:128, 0:1])
                        else:
                            nc.scalar.activation(qg_sb[64:128, :],
                                                 pqg[64:128, :], Copy)

                    # v in natural [s, d] layout: lhsT = x s-block
                    pv = pvp.tile([128, 4, 128], F32, tag="pv")
                    for sb in range(4):
                        sb0 = sb * 128
                        for ft in range(FT):
                            nc.tensor.matmul(pv[:, sb, :],
                                             xkv_t[:, ft, sb0:sb0 + 128],
                                             wv_sb[:, ft, :],
                                             start=(ft == 0), stop=(ft == FT - 1))
                    kt0 = ss // 128
                    nc.vector.tensor_copy(vsw[:, kt0:kt0 + 4, 0:HD],
                                          pv[:, :, 0:HD])
                    nc.vector.tensor_copy(vg[:, kt0:kt0 + 4, 0:HD],
                                          pv[:, :, HD:2 * HD])

            # ====== Phases B + C: global + sliding-window attention ======
            # Phase C is a 3-stage software pipeline; phase B's work is
            # injected into the pipeline drain steps, reusing C's psum pools.
            GG = 512 // ng if ng > 0 else 1   # B ktiles per psum bank
            NGRP = NKT // GG
            with (
                tc.tile_pool(name="E", bufs=4) as ep,
                tc.tile_pool(name="xt", bufs=3) as xtp,
                tc.tile_pool(name="osb", bufs=2) as osbp,
                tc.tile_pool(name="rc", bufs=2) as rcp,
                tc.tile_pool(name="gx", bufs=1) as gxp,
                tc.tile_pool(name="pL", bufs=2, space="PSUM") as pLp,
                tc.tile_pool(name="pX", bufs=2, space="PSUM") as pXp,
                tc.tile_pool(name="pO", bufs=1, space="PSUM") as pOp,
            ):
                Es = {}
                Egs = {}
                xts = {}
                eg = gxp.tile([128, NKT, max(ng, 1)], F16, tag="eg")
                Bst = {"pxg": None}

                def stage1(t):
                    qs = t * ST
                    j0, j1, ml_present, mr_off = _sw_tiles(t)
                    nkt = j1 - j0
                    has_g = ng > 0 and j0 > 0
                    E = ep.tile([128, 6 * ST], F16, tag="E")
                    Es[t] = E
                    # global-key prepend first: its small exp frees the pL
                    # slot early, keeping the pool stall-free at bufs=2
                    if has_g:
                        plg2 = pLp.tile([ng, ST], F32, tag="L")
                        nc.tensor.matmul(plg2[:], kT[:, 0:ng],
                                         qT[:, qs:qs + ST],
                                         start=True, stop=True)
                        Eg2 = ep.tile([ng, ST], F16, tag="Eg")
                        nc.scalar.activation(Eg2[:], plg2[:], Exp, scale=0.125)
                        Egs[t] = Eg2
                    # 3-ktile groups (two ACT calls per supertile)
                    for a in range(0, nkt, 3):
                        b = min(a + 3, nkt)
                        pl = pLp.tile([128, (b - a) * ST], F32, tag="L")
                        for s in range(b - a):
                            j = j0 + a + s
                            nc.tensor.matmul(pl[:, s * ST:(s + 1) * ST],
                                             kT[:, j * KT:(j + 1) * KT],
                                             qT[:, qs:qs + ST],
                                             start=True, stop=True)
                        nc.scalar.activation(E[:, a * ST:b * ST], pl[:], Exp,
                                             scale=0.125)
                    # masks (ML on gpsimd, MR on vector to balance engines)
                    if ml_present:
                        msk = mlg_sb if (t == 1 and ng > 0) else ml_sb
                        nc.gpsimd.tensor_mul(E[:, 0:2 * ST], E[:, 0:2 * ST],
                                             msk[:])
                    if mr_off is not None:
                        nc.vector.tensor_mul(E[:, mr_off:mr_off + 2 * ST],
                                             E[:, mr_off:mr_off + 2 * ST],
                                             mr_sb[:])

                def stage2(t):
                    j0, j1, _, _ = _sw_tiles(t)
                    nkt = j1 - j0
                    has_g = ng > 0 and j0 > 0
                    E = Es.pop(t)
                    # AV: xT' = [v|1].T @ expw.T -> [65, 256], sums in row 64
                    px = pXp.tile([HD + 1, ST], F32, tag="X")
                    for s in range(nkt):
                        j = j0 + s
                        nc.tensor.matmul(px[:], vsw[:, j, :],
                                         E[:, s * ST:(s + 1) * ST],
                                         start=(s == 0),
                                         stop=(s == nkt - 1 and not has_g))
                    if has_g:
                        nc.tensor.matmul(px[:], vsw[0:ng, 0, :], Egs.pop(t),
                                         start=False, stop=True)
                    xT = xtp.tile([HD + 1, ST], F16, tag="xT")
                    nc.vector.tensor_copy(xT[:], px[:])
                    xts[t] = xT

                def stage3(t):
                    xT = xts.pop(t)
                    # denominators (row 64) -> per-partition columns
                    ps = pXp.tile([128, 2], F32, tag="X")
                    for hf in range(2):
                        nc.tensor.matmul(ps[:, hf:hf + 1],
                                         xT[HD:HD + 1, hf * 128:(hf + 1) * 128],
                                         one_sb[HD:HD + 1, 0:1],
                                         start=True, stop=True)
                    rc = rcp.tile([128, 2], F32, tag="rc")
                    nc.vector.reciprocal(rc[:], ps[:])
                    po = pOp.tile([128, 2, F], F32, tag="O")
                    for hf in range(2):
                        nc.tensor.matmul(po[:, hf, :],
                                         xT[0:HD, hf * 128:(hf + 1) * 128],
                                         wo_sb[:], start=True, stop=True)
                    osb = osbp.tile([128, 2, F], F16, tag="osb")
                    # normalization folded into the evacuation copies
                    nc.vector.tensor_scalar_mul(osb[:, 0, :], po[:, 0, :],
                                                rc[:, 0:1])
                    nc.vector.tensor_scalar_mul(osb[:, 1, :], po[:, 1, :],
                                                rc[:, 1:2])
                    if t == 0 and ng > 0:
                        nc.sync.dma_start(out_ap[ng:128, :], osb[ng:128, 0, :])
                        nc.sync.dma_start(out_ap[128:256, :], osb[:, 1, :])
                    else:
                        nc.sync.dma_start(
                            out_ap.rearrange("(a p) f -> p a f", p=128)
                            [:, 2 * t:2 * t + 2, :], osb[:])

                def b_group(grp):
                    # global-branch logits+exp for ktile group grp, and AV
                    # for group grp-1 (pipelined)
                    if grp < NGRP:
                        plg = pLp.tile([128, GG * ng], F32, tag="L")
                        for j in range(GG):
                            kt = grp * GG + j
                            nc.tensor.matmul(plg[:, j * ng:(j + 1) * ng],
                                             kT[64:128, kt * KT:(kt + 1) * KT],
                                             qg_sb[64:128, 0:ng],
                                             start=True, stop=True)
                        nc.scalar.activation(
                            eg[:, grp * GG:(grp + 1) * GG, :], plg[:], Exp,
                            scale=0.125)
                    if grp >= 1:
                        if Bst["pxg"] is None:
                            Bst["pxg"] = pXp.tile([HD + 1, ng], F32, tag="X",
                                                  name="pxg")
                        pxg = Bst["pxg"]
                        for j in range(GG):
                            kt = (grp - 1) * GG + j
                            nc.tensor.matmul(pxg[:], vg[:, kt, :],
                                             eg[:, kt, :],
                                             start=(kt == 0),
                                             stop=(kt == NKT - 1))

                def b_finish():
                    pxg = Bst["pxg"]
                    xgT = gxp.tile([HD + 1, ng], F16, tag="xgT")
                    nc.vector.tensor_copy(xgT[:], pxg[:])
                    psg = pXp.tile([ng, 1], F32, tag="X")
                    nc.tensor.matmul(psg[:], xgT[HD:HD + 1, 0:ng],
                                     one_sb[HD:HD + 1, 0:1],
                                     start=True, stop=True)
                    rg = gxp.tile([ng, 1], F32, tag="rg")
                    nc.vector.reciprocal(rg[:], psg[:])
                    pog = pOp.tile([ng, F], F32, tag="O")
                    nc.tensor.matmul(pog[:], xgT[0:HD, 0:ng], wo_sb[:],
                                     start=True, stop=True)
                    og = gxp.tile([ng, F], F16, tag="og_sb")
                    nc.scalar.activation(og[:], pog[:], Copy, scale=rg[:, 0:1])
                    nc.sync.dma_start(out_ap[0:ng, :], og[:])

                # B group g is issued at step NST + g (the pipeline drain);
                # b_finish after the last B AV.
                NSTEP = max(NST + 2, NST + NGRP + 2) if ng > 0 else NST + 2
                for step in range(NSTEP):
                    if step < NST:
                        stage1(step)
                    if 1 <= step <= NST:
                        stage2(step - 1)
                    if step >= 2 and step - 2 < NST:
                        stage3(step - 2)
                    if ng > 0 and NST <= step <= NST + NGRP:
                        b_group(step - NST)
                        if step == NST + NGRP:
                            b_finish()

    nc.compile()
    return nc


@functools.lru_cache(maxsize=4)
def _get_program(ng: int, use_bq: bool):
    return _build_program(ng, use_bq)


def kernel(inputs_q, inputs_kv, global_mask,
           w_q_sw, b_q_sw, w_k_sw, b_k_sw, w_v_sw, b_v_sw,
           w_q_g, b_q_g, w_k_g, b_k_g, w_v_g, b_v_g,
           w_out, b_out,
           _trace=False, _tmpdir=None):
    gm = np.asarray(global_mask[0]).astype(bool)
    ng = int(gm.sum())
    assert gm[:ng].all() and not gm[ng:].any(), "global_mask must be a prefix mask"
    assert ng <= NQG, "kernel specialized for ng <= 128"
    assert ng == 0 or 512 % ng == 0, "kernel requires ng dividing 512"

    def _prep(x):
        # [S, F] -> xT [F, S] -> [128, NSC=8, FT=4, SC=512] flat per partition
        xT = np.asarray(x, np.float32).T.reshape(4, 128, 8, 512)
        return np.ascontiguousarray(xT.transpose(1, 2, 0, 3)
                                    ).reshape(128, 4 * S).astype(NP16)

    xqT = _prep(inputs_q[0])
    xkvT = _prep(inputs_kv[0])
    ml, mr, mlg = _build_masks(ng)

    use_bq = bool(np.any(b_q_sw) or np.any(b_q_g))
    nc = _get_program(ng, use_bq)

    zeros_hd = np.zeros((F, HD), np.float32)
    in_maps = []
    for h in range(N_CORES):
        wq = np.concatenate([w_q_sw[:, h, :], zeros_hd], axis=1).astype(NP16)
        wk = np.concatenate([w_k_sw[:, h, :], w_k_g[:, h, :]], axis=1).astype(NP16)
        wv = np.concatenate([w_v_sw[:, h, :], w_v_g[:, h, :]], axis=1).astype(NP16)
        wqg = np.ascontiguousarray(w_q_g[:, h, :]).astype(NP16)
        bq = np.concatenate([b_q_sw[h], np.zeros((HD,), np.float32)]
                            ).reshape(2 * HD, 1).astype(np.float32)
        bqg = np.asarray(b_q_g[h]).reshape(HD, 1).astype(np.float32)
        wo = np.asarray(w_out[h], NP16)
        in_maps.append({
            "xqT": xqT, "xkvT": xkvT,
            "wq": wq, "wk": wk, "wv": wv, "wqg": wqg,
            "bq": bq, "bqg": bqg,
            "wo": wo, "ml": ml, "mr": mr, "mlg": mlg,
        })

    res = run_bass_kernel_spmd(nc, in_maps, list(range(N_CORES)),
                               trace=_trace, tmpdir=_tmpdir)
    partial = np.stack([res.results[h]["out"] for h in range(N_CORES)])
    out = partial.astype(np.float32).sum(axis=0)

    # v-bias correction: softmax weights sum to 1, so a v bias adds
    # (b_v @ w_out) summed over heads -- a constant row per branch.
    b_base = np.asarray(b_out, np.float32)
    b_sw = b_base + np.einsum("hd,hdf->f", np.asarray(b_v_sw, np.float32),
                              np.asarray(w_out, np.float32))
    b_g = b_base + np.einsum("hd,hdf->f", np.asarray(b_v_g, np.float32),
                             np.asarray(w_out, np.float32))
    out += b_sw[None, :]
    if ng > 0:
        out[:ng] += (b_g - b_sw)[None, :]
    if _trace:
        kernel._last_results = res
    return out[None].astype(np.float32)
